# revision 32
# baseline (speedup 1.0000x reference)
"""Causal self-attention kernel for Trainium2, 8 NeuronCores.

Sharding: core c handles batch b = c//2 and head-half c%2 (8 of 16 heads,
512 of 1024 features). Tensor-parallel style: Wq/Wk/Wv split column-wise,
Wp split row-wise; the 2 cores of a batch produce partial outputs that the
host sums (plus the bias const row).

Per-core device program (identical across cores, data differs):
  - projections: qT/kT in [feature, t] layout, v in [s, feature] layout
  - per head-pair (even head on SBUF partitions 0-63, odd head on 64-127 so
    their K=64 score matmuls occupy disjoint PE row-groups and overlap):
    S^T[s, t] = k_s . q_t scores via PE, exp via ACT with fused 1/sqrt(d)
    scale and key-mask bias; causal handled by ragged matmuls plus one
    triangular-mask multiply on the diagonal 128x128 block per chunk.
    Work is tiled as (s-tile 128) x (t-chunk 512); exp rows are stored in
    two 512-wide slots (slot = tb % 2), freed by the att@v burst two
    t-chunks later, so both heads' live scores fit in SBUF.
  - att@v with a ones-column appended to v so the softmax denominator
    accumulates as row 64 of the PSUM tile; normalize by broadcast multiply
    (gpsimd partition_broadcast of the reciprocal row)
  - output projection from the transposed y layout (natural [t, j] output)
"""

import sys

sys.path.insert(0, "/opt/trn_rl_repo")

import numpy as np
import ml_dtypes

import concourse.bass as bass
import concourse.mybir as mybir
import concourse.tile as tile
from concourse import bacc
from concourse.bass_utils import run_bass_kernel_spmd

B, T, C, H = 4, 2048, 1024, 16
D = 64          # head dim
NCORES = 8
NF = 512        # features per core (8 heads)
NH = 8          # heads per core
CT = C // 128   # 8 contraction chunks
NTB = T // 512  # 4 t-blocks
NST = T // 128  # 16 s-tiles
F32 = mybir.dt.float32
BF16 = mybir.dt.bfloat16
BF16NP = ml_dtypes.bfloat16

_NC_CACHE = {}


def _build():
    nc = bacc.Bacc("TRN2", target_bir_lowering=False, debug=False,
                   num_devices=NCORES)
    xT = nc.dram_tensor("xT", [C, T], BF16, kind="ExternalInput")
    wqT = nc.dram_tensor("wqT", [C, NF], BF16, kind="ExternalInput")
    wkT = nc.dram_tensor("wkT", [C, NF], BF16, kind="ExternalInput")
    wvT = nc.dram_tensor("wvT", [C, NF], BF16, kind="ExternalInput")
    wpT = nc.dram_tensor("wpT", [NF, C], BF16, kind="ExternalInput")
    kmask = nc.dram_tensor("kmask", [T], F32, kind="ExternalInput")
    bqv = nc.dram_tensor("bqv", [NF], F32, kind="ExternalInput")
    bkv = nc.dram_tensor("bkv", [NF], F32, kind="ExternalInput")
    tri = nc.dram_tensor("tri", [128, 256], BF16, kind="ExternalInput")
    part = nc.dram_tensor("part", [T, C], F32, kind="ExternalOutput")

    EXP = mybir.ActivationFunctionType.Exp
    SCALE = 1.0 / 8.0  # 1/sqrt(D)

    with tile.TileContext(nc) as tc:
        with (
            tc.tile_pool(name="const", bufs=1) as const,
            tc.tile_pool(name="small", bufs=3) as small, \
            tc.tile_pool(name="obp", bufs=4) as obp,
            tc.tile_pool(name="pp_s", bufs=2, space="PSUM") as pp_s,
            tc.tile_pool(name="pp_y", bufs=4, space="PSUM") as pp_y,
        ):
            # ---- persistent tiles ----
            v_sb = const.tile([128, NST, NH, D + 1], BF16)  # [s_loc, st, h, 65]
            yT_sb = const.tile([128, 4, T], BF16)   # [p, jt, t]
            wp_sb = const.tile([128, 4, C], BF16)   # [p, cj, j]
            tri_sb = const.tile([128, 2, 128], BF16)
            kb_sb = const.tile([128, NST], F32)     # [s_loc, st]
            bq_sb = const.tile([128, 4], F32)
            bk_sb = const.tile([128, 4], F32)

            # ones column for the softmax denominator trick
            nc.vector.memset(v_sb[:, :, :, D:D + 1], 1.0)

            with tc.tile_pool(name="proj", bufs=1) as projp, \
                 tc.tile_pool(name="wjt", bufs=4) as wjtp, \
                 tc.tile_pool(name="qkp", bufs=2) as qkp, \
                 tc.tile_pool(name="wvp", bufs=1) as wvp, \
                 tc.tile_pool(name="expp", bufs=1) as expp:

                def load_wjt(wT, jt, nm):
                    w_jt = wjtp.tile([128, CT, 128], BF16, tag="wjt", name=nm)
                    for ci in range(CT):
                        nc.sync.dma_start(
                            out=w_jt[:, ci, :],
                            in_=wT[ci * 128:(ci + 1) * 128,
                                   jt * 128:(jt + 1) * 128])
                    return w_jt

                x_sb = projp.tile([128, CT, T], BF16)
                for ci in range(2):
                    nc.sync.dma_start(out=x_sb[:, ci, 0:1024],
                                      in_=xT[ci * 128:(ci + 1) * 128, 0:1024])
                wq_jt = load_wjt(wqT, 0, "wq0")
                wk_jt = load_wjt(wkT, 0, "wk0")
                for ci in range(2, CT):
                    nc.sync.dma_start(out=x_sb[:, ci, 0:1024],
                                      in_=xT[ci * 128:(ci + 1) * 128, 0:1024])
                for ci in range(CT):
                    nc.sync.dma_start(out=x_sb[:, ci, 1024:T],
                                      in_=xT[ci * 128:(ci + 1) * 128, 1024:T])
                nc.sync.dma_start(out=tri_sb,
                                  in_=tri.rearrange("p (u q) -> p u q", u=2))
                nc.sync.dma_start(out=kb_sb,
                                  in_=bass.AP(tensor=kmask, offset=0,
                                              ap=[[1, 128], [128, NST]]))
                nc.sync.dma_start(out=bq_sb,
                                  in_=bass.AP(tensor=bqv, offset=0,
                                              ap=[[1, 128], [128, 4]]))
                nc.sync.dma_start(out=bk_sb,
                                  in_=bass.AP(tensor=bkv, offset=0,
                                              ap=[[1, 128], [128, 4]]))
                wv_sb = wvp.tile([128, CT, NF], BF16)
                for ci in range(CT):
                    nc.sync.dma_start(out=wv_sb[:, ci, :],
                                      in_=wvT[ci * 128:(ci + 1) * 128, :])

                # [s_loc, head_parity, sb, t_within_phase]
                exp_sb = expp.tile([128, 2, NST, 1024], BF16)

                def qk_chain(w_jt, dst, b_sb, jt, tb):
                    ps = pp_y.tile([128, 512], F32, tag="py", name=f"q{jt}{tb}")
                    for ci in range(CT):
                        nc.tensor.matmul(
                            ps,
                            lhsT=w_jt[:, ci, :],
                            rhs=x_sb[:, ci, tb * 512:(tb + 1) * 512],
                            start=(ci == 0), stop=(ci == CT - 1))
                    nc.vector.tensor_scalar_add(
                        dst[:, tb * 512:(tb + 1) * 512], ps,
                        b_sb[:, jt:jt + 1])

                def v_group(st0, st1):
                    # v: natural [s, feature] layout (bv folded in on host)
                    for st in range(st0, st1):
                        ps = pp_y.tile([128, 512], F32, tag="py", name=f"v{st}")
                        for ci in range(CT):
                            nc.tensor.matmul(
                                ps,
                                lhsT=x_sb[:, ci, st * 128:(st + 1) * 128],
                                rhs=wv_sb[:, ci, :],
                                start=(ci == 0), stop=(ci == CT - 1))
                        nc.vector.tensor_copy(
                            out=v_sb[:, st, :, 0:D],
                            in_=ps.rearrange("p (h d) -> p h d", h=NH))

                def burst(hp, h, tb):
                    """att @ v for head h, t-block tb (rows 0..4tb+3 ready)."""
                    hh = h % 2
                    tlo = tb * 512
                    slot = (tb % 2) * 512
                    y_ps = pp_y.tile([128, 512], F32, tag="py", name=f"y{h}{tb}")
                    smax = 4 * (tb + 1)
                    for i in range(smax):
                        t0 = max(tlo, i * 128)
                        nc.tensor.matmul(
                            y_ps[0:D + 1, t0 - tlo:512],
                            lhsT=v_sb[:, i, h, :],
                            rhs=exp_sb[:, hh, i, slot + t0 - tlo:slot + 512],
                            start=(i == 0), stop=(i == smax - 1))
                    rrow = small.tile([1, 512], F32, tag="rrow")
                    nc.vector.reciprocal(rrow, y_ps[D:D + 1, :])
                    bcast = small.tile([64, 512], F32, tag="bcast")
                    nc.gpsimd.partition_broadcast(bcast, rrow, channels=64)
                    dst_sl = slice(tlo, tlo + 512)
                    if hh == 0:
                        nc.vector.tensor_mul(
                            yT_sb[0:64, hp, dst_sl], y_ps[0:D, :], bcast)
                    else:
                        tmp = small.tile([64, 512], BF16, tag="odd")
                        nc.vector.tensor_mul(tmp, y_ps[0:D, :], bcast)
                        nc.sync.dma_start(
                            out=yT_sb[64:128, hp, dst_sl], in_=tmp)

                def schunk(hp, qT_t, kT_t, sb, tb):
                    """score chunk [s-tile sb] x [t-block tb], both heads in
                    one 2-bank psum tile; one fused exp ACTIVATE."""
                    hA, hB = 2 * hp, 2 * hp + 1
                    q_ = {hA: qT_t[0:64, :], hB: qT_t[64:128, :]}
                    k_ = {hA: kT_t[0:64, :], hB: kT_t[64:128, :]}
                    s0, tlo = sb * 128, tb * 512
                    t0 = max(s0, tlo)
                    off = t0 - tlo
                    slot = (tb % 2) * 512
                    ps = pp_s.tile([128, 2, 512], F32, tag="ps",
                                   name=f"s{sb}_{tb}")
                    for h in (hA, hB):  # disjoint PE row-groups: overlap on HW
                        nc.tensor.matmul(
                            ps[:, h % 2, off:512],
                            lhsT=k_[h][:, s0:s0 + 128],
                            rhs=q_[h][:, t0:tlo + 512],
                            start=True, stop=True)
                    nc.scalar.activation(
                        exp_sb[:, :, sb, slot + off:slot + 512],
                        ps[:, :, off:512],
                        EXP, bias=kb_sb[:, sb:sb + 1], scale=SCALE)
                    if s0 >= tlo:  # diagonal 128-block: causal triangle mask
                        dg = exp_sb[:, :, sb, slot + off:slot + off + 128]
                        nc.vector.tensor_mul(dg, dg, tri_sb)

                def pair(hp, ctx, nxt):
                    # ctx = (wq_jt, wk_jt, qT_t, kT_t); the tb0 q/k chains
                    # were already emitted (previous pair's tb3 prologue or
                    # the caller for pair 0). nxt = next pair's ctx.
                    hA, hB = 2 * hp, 2 * hp + 1
                    wq_jt, wk_jt, qT_t, kT_t = ctx
                    for tb in range(NTB):
                        if tb > 0:
                            qk_chain(wq_jt, qT_t, bq_sb, hp, tb)
                            qk_chain(wk_jt, kT_t, bk_sb, hp, tb)
                        if tb == 3 and nxt is not None:
                            # next pair's first chains fill our tail
                            qk_chain(nxt[0], nxt[2], bq_sb, hp + 1, 0)
                            qk_chain(nxt[1], nxt[3], bk_sb, hp + 1, 0)
                        for sb in range(4 * tb + 4):
                            schunk(hp, qT_t, kT_t, sb, tb)
                        if hp == 0:
                            v_group(4 * tb, 4 * tb + 4)
                        burst(hp, hA, tb)
                        burst(hp, hB, tb)
                        if hp == 3:
                            out_proj(4 * tb, 4 * tb + 4)

                def out_proj(tt0, tt1):
                    # output projection for t-rows [tt0*128, tt1*128);
                    # emitted right after the pair-3 burst that completes
                    # those yT rows, so it overlaps the rest of pair 3
                    for tt in range(tt0, tt1):
                        for jb in range(2):
                            ps = pp_y.tile([128, 512], F32, tag="py",
                                           name=f"o{tt}{jb}")
                            for cj in range(4):
                                nc.tensor.matmul(
                                    ps,
                                    lhsT=yT_sb[:, cj, tt * 128:(tt + 1) * 128],
                                    rhs=wp_sb[:, cj, jb * 512:(jb + 1) * 512],
                                    start=(cj == 0), stop=(cj == 3))
                            ob = obp.tile([128, 512], F32, tag="ob")
                            nc.vector.tensor_copy(out=ob, in_=ps)
                            nc.sync.dma_start(
                                out=part[tt * 128:(tt + 1) * 128,
                                        jb * 512:(jb + 1) * 512],
                                in_=ob)

                def make_ctx(hp):
                    return (load_wjt(wqT, hp, f"wq{hp}"),
                            load_wjt(wkT, hp, f"wk{hp}"),
                            qkp.tile([128, T], BF16, tag="qT", name=f"qT{hp}"),
                            qkp.tile([128, T], BF16, tag="kT", name=f"kT{hp}"))

                ctx = (wq_jt, wk_jt,
                       qkp.tile([128, T], BF16, tag="qT", name="qT0"),
                       qkp.tile([128, T], BF16, tag="kT", name="kT0"))
                qk_chain(ctx[0], ctx[2], bq_sb, 0, 0)
                qk_chain(ctx[1], ctx[3], bk_sb, 0, 0)
                nxt = make_ctx(1)
                pair(0, ctx, nxt)
                for cj in range(4):
                    nc.sync.dma_start(out=wp_sb[:, cj, :],
                                      in_=wpT[cj * 128:(cj + 1) * 128, :])
                for hp in range(1, 4):
                    ctx = nxt
                    nxt = make_ctx(hp + 1) if hp < 3 else None
                    pair(hp, ctx, nxt)

    nc.compile()
    return nc


def _get_nc():
    if "nc" not in _NC_CACHE:
        _NC_CACHE["nc"] = _build()
    return _NC_CACHE["nc"]


def _make_in_maps(x, mask, Wq, bq, Wk, bk, Wv, bv, Wp, bp):
    tri1 = np.triu(np.ones((128, 128), dtype=BF16NP))  # keep s <= t
    tri = np.concatenate([tri1, tri1], axis=1)
    xTs = [np.ascontiguousarray(x[b].T).astype(BF16NP) for b in range(B)]
    kms = [((1.0 - mask[b]) * -10000.0).astype(np.float32) for b in range(B)]
    halves = []
    for half in range(2):
        F = slice(half * NF, half * NF + NF)
        halves.append({
            "wqT": np.ascontiguousarray(Wq[F, :].T).astype(BF16NP),
            "wkT": np.ascontiguousarray(Wk[F, :].T).astype(BF16NP),
            "wvT": np.ascontiguousarray(Wv[F, :].T).astype(BF16NP),
            "wpT": np.ascontiguousarray(Wp[:, F].T).astype(BF16NP),
            "bqv": bq[F].astype(np.float32).copy(),
            "bkv": bk[F].astype(np.float32).copy(),
            "tri": tri,
        })
    return [{"xT": xTs[c // 2], "kmask": kms[c // 2], **halves[c % 2]}
            for c in range(NCORES)]


def kernel(x, mask, Wq, bq, Wk, bk, Wv, bv, Wp, bp):
    x = np.asarray(x, dtype=np.float32)
    mask = np.asarray(mask, dtype=np.float32)
    Wq, bq = np.asarray(Wq, np.float32), np.asarray(bq, np.float32)
    Wk, bk = np.asarray(Wk, np.float32), np.asarray(bk, np.float32)
    Wv, bv = np.asarray(Wv, np.float32), np.asarray(bv, np.float32)
    Wp, bp = np.asarray(Wp, np.float32), np.asarray(bp, np.float32)

    nc = _get_nc()
    in_maps = _make_in_maps(x, mask, Wq, bq, Wk, bk, Wv, bv, Wp, bp)
    res = run_bass_kernel_spmd(nc, in_maps, list(range(NCORES)))
    const_row = (bv @ Wp.T + bp).astype(np.float32)  # [C]
    out = np.empty((B, T, C), np.float32)
    for b in range(B):
        out[b] = res.results[2 * b]["part"] + res.results[2 * b + 1]["part"]
        out[b] += const_row
    return out


# revision 39
# speedup vs baseline: 1.0801x; 1.0801x over previous
"""Causal self-attention kernel for Trainium2, 8 NeuronCores.

Sharding: core c handles batch b = c//2 and head-half c%2 (8 of 16 heads,
512 of 1024 features). Tensor-parallel style: Wq/Wk/Wv split column-wise,
Wp split row-wise; the 2 cores of a batch produce partial outputs that the
host sums (plus the bias const row).

Per-core device program (identical across cores, data differs):
  - projections: qT/kT in [feature, t] layout, v in [s, feature] layout
  - per head-pair (even head on SBUF partitions 0-63, odd head on 64-127 so
    their K=64 score matmuls occupy disjoint PE row-groups and overlap):
    S^T[s, t] = k_s . q_t scores via PE, exp via ACT with fused 1/sqrt(d)
    scale and key-mask bias; causal handled by ragged matmuls plus one
    triangular-mask multiply on the diagonal 128x128 block per chunk.
    Work is tiled as (s-tile 128) x (t-chunk 512); exp rows are stored in
    two 512-wide slots (slot = tb % 2), freed by the att@v burst two
    t-chunks later, so both heads' live scores fit in SBUF.
  - att@v with a ones-column appended to v so the softmax denominator
    accumulates as row 64 of the PSUM tile; normalize by broadcast multiply
    (gpsimd partition_broadcast of the reciprocal row)
  - output projection from the transposed y layout (natural [t, j] output)
"""

import sys

sys.path.insert(0, "/opt/trn_rl_repo")

import numpy as np
import ml_dtypes

import concourse.bass as bass
import concourse.mybir as mybir
import concourse.tile as tile
from concourse import bacc
from concourse.bass_utils import run_bass_kernel_spmd

B, T, C, H = 4, 2048, 1024, 16
D = 64          # head dim
NCORES = 8
NF = 512        # features per core (8 heads)
NH = 8          # heads per core
CT = C // 128   # 8 contraction chunks
NTB = T // 512  # 4 t-blocks
NST = T // 128  # 16 s-tiles
F32 = mybir.dt.float32
BF16 = mybir.dt.bfloat16
BF16NP = ml_dtypes.bfloat16

_NC_CACHE = {}


def _build():
    nc = bacc.Bacc("TRN2", target_bir_lowering=False, debug=False,
                   num_devices=NCORES)
    xT = nc.dram_tensor("xT", [C, T], BF16, kind="ExternalInput")
    wqT = nc.dram_tensor("wqT", [C, NF], BF16, kind="ExternalInput")
    wkT = nc.dram_tensor("wkT", [C, NF], BF16, kind="ExternalInput")
    wvT = nc.dram_tensor("wvT", [C, NF], BF16, kind="ExternalInput")
    wpT = nc.dram_tensor("wpT", [NF, C], BF16, kind="ExternalInput")
    kmask = nc.dram_tensor("kmask", [T], F32, kind="ExternalInput")
    bqv = nc.dram_tensor("bqv", [NF], F32, kind="ExternalInput")
    bkv = nc.dram_tensor("bkv", [NF], F32, kind="ExternalInput")
    tri = nc.dram_tensor("tri", [128, 256], BF16, kind="ExternalInput")
    part = nc.dram_tensor("part", [T, C], F32, kind="ExternalOutput")

    EXP = mybir.ActivationFunctionType.Exp
    SCALE = 1.0 / 8.0  # 1/sqrt(D)

    with tile.TileContext(nc) as tc:
        with (
            tc.tile_pool(name="const", bufs=1) as const,
            tc.tile_pool(name="small", bufs=3) as small, \
            tc.tile_pool(name="obp", bufs=4) as obp,
            tc.tile_pool(name="pp_s", bufs=2, space="PSUM") as pp_s,
            tc.tile_pool(name="pp_y", bufs=4, space="PSUM") as pp_y,
        ):
            # ---- persistent tiles ----
            v_sb = const.tile([128, NST, NH, D + 1], BF16)  # [s_loc, st, h, 65]
            yT_sb = const.tile([128, 4, T], BF16)   # [p, jt, t]
            wp_sb = const.tile([128, 4, C], BF16)   # [p, cj, j]
            tri_sb = const.tile([128, 2, 128], BF16)
            kb_sb = const.tile([128, NST], F32)     # [s_loc, st]
            bq_sb = const.tile([128, 4], F32)
            bk_sb = const.tile([128, 4], F32)

            # ones column for the softmax denominator trick
            nc.vector.memset(v_sb[:, :, :, D:D + 1], 1.0)

            with tc.tile_pool(name="proj", bufs=1) as projp, \
                 tc.tile_pool(name="wjt", bufs=4) as wjtp, \
                 tc.tile_pool(name="qkp", bufs=2) as qkp, \
                 tc.tile_pool(name="wvp", bufs=1) as wvp, \
                 tc.tile_pool(name="expp", bufs=1) as expp:

                def load_wjt(wT, jt, nm):
                    w_jt = wjtp.tile([128, CT, 128], BF16, tag="wjt", name=nm)
                    for ci in range(CT):
                        nc.sync.dma_start(
                            out=w_jt[:, ci, :],
                            in_=wT[ci * 128:(ci + 1) * 128,
                                   jt * 128:(jt + 1) * 128])
                    return w_jt

                x_sb = projp.tile([128, CT, T], BF16)
                wq_jt = load_wjt(wqT, 0, "wq0")
                wk_jt = load_wjt(wkT, 0, "wk0")
                for ci in range(2):
                    eng = nc.sync if ci % 2 == 0 else nc.scalar
                    eng.dma_start(out=x_sb[:, ci, 0:1024],
                                  in_=xT[ci * 128:(ci + 1) * 128, 0:1024])
                for ci in range(2, CT):
                    eng = nc.sync if ci % 2 == 0 else nc.scalar
                    eng.dma_start(out=x_sb[:, ci, 0:1024],
                                  in_=xT[ci * 128:(ci + 1) * 128, 0:1024])
                for ci in range(CT):
                    nc.sync.dma_start(out=x_sb[:, ci, 1024:T],
                                      in_=xT[ci * 128:(ci + 1) * 128, 1024:T])
                nc.sync.dma_start(out=tri_sb,
                                  in_=tri.rearrange("p (u q) -> p u q", u=2))
                nc.sync.dma_start(out=kb_sb,
                                  in_=bass.AP(tensor=kmask, offset=0,
                                              ap=[[1, 128], [128, NST]]))
                nc.sync.dma_start(out=bq_sb,
                                  in_=bass.AP(tensor=bqv, offset=0,
                                              ap=[[1, 128], [128, 4]]))
                nc.sync.dma_start(out=bk_sb,
                                  in_=bass.AP(tensor=bkv, offset=0,
                                              ap=[[1, 128], [128, 4]]))
                wv_sb = wvp.tile([128, CT, NF], BF16)
                for ci in range(CT):
                    nc.sync.dma_start(out=wv_sb[:, ci, :],
                                      in_=wvT[ci * 128:(ci + 1) * 128, :])

                # [s_loc, head_parity, sb, t_within_phase]
                exp_sb = expp.tile([128, 2, NST, 1024], BF16)

                def qk_chain(w_jt, dst, b_sb, jt, tb):
                    ps = pp_y.tile([128, 512], F32, tag="py", name=f"q{jt}{tb}")
                    for ci in range(CT):
                        nc.tensor.matmul(
                            ps,
                            lhsT=w_jt[:, ci, :],
                            rhs=x_sb[:, ci, tb * 512:(tb + 1) * 512],
                            start=(ci == 0), stop=(ci == CT - 1))
                    nc.vector.tensor_scalar_add(
                        dst[:, tb * 512:(tb + 1) * 512], ps,
                        b_sb[:, jt:jt + 1])

                def v_group(st0, st1):
                    # v: natural [s, feature] layout (bv folded in on host)
                    for st in range(st0, st1):
                        ps = pp_y.tile([128, 512], F32, tag="py", name=f"v{st}")
                        for ci in range(CT):
                            nc.tensor.matmul(
                                ps,
                                lhsT=x_sb[:, ci, st * 128:(st + 1) * 128],
                                rhs=wv_sb[:, ci, :],
                                start=(ci == 0), stop=(ci == CT - 1))
                        nc.any.tensor_copy(
                            out=v_sb[:, st, :, 0:D],
                            in_=ps.rearrange("p (h d) -> p h d", h=NH))

                def burst(hp, h, tb):
                    """att @ v for head h, t-block tb (rows 0..4tb+3 ready)."""
                    hh = h % 2
                    tlo = tb * 512
                    slot = (tb % 2) * 512
                    y_ps = pp_y.tile([128, 512], F32, tag="py", name=f"y{h}{tb}")
                    smax = 4 * (tb + 1)
                    for i in range(smax):
                        t0 = max(tlo, i * 128)
                        nc.tensor.matmul(
                            y_ps[0:D + 1, t0 - tlo:512],
                            lhsT=v_sb[:, i, h, :],
                            rhs=exp_sb[:, hh, i, slot + t0 - tlo:slot + 512],
                            start=(i == 0), stop=(i == smax - 1))
                    rrow = small.tile([1, 512], F32, tag="rrow")
                    nc.vector.reciprocal(rrow, y_ps[D:D + 1, :])
                    bcast = small.tile([64, 512], F32, tag="bcast")
                    nc.gpsimd.partition_broadcast(bcast, rrow, channels=64)
                    dst_sl = slice(tlo, tlo + 512)
                    if hh == 0:
                        nc.vector.tensor_mul(
                            yT_sb[0:64, hp, dst_sl], y_ps[0:D, :], bcast)
                    else:
                        tmp = small.tile([64, 512], BF16, tag="odd")
                        nc.vector.tensor_mul(tmp, y_ps[0:D, :], bcast)
                        nc.sync.dma_start(
                            out=yT_sb[64:128, hp, dst_sl], in_=tmp)

                def schunk(hp, qT_t, kT_t, sb, tb):
                    """score chunk [s-tile sb] x [t-block tb], both heads in
                    one 2-bank psum tile; one fused exp ACTIVATE."""
                    hA, hB = 2 * hp, 2 * hp + 1
                    q_ = {hA: qT_t[0:64, :], hB: qT_t[64:128, :]}
                    k_ = {hA: kT_t[0:64, :], hB: kT_t[64:128, :]}
                    s0, tlo = sb * 128, tb * 512
                    t0 = max(s0, tlo)
                    off = t0 - tlo
                    slot = (tb % 2) * 512
                    ps = pp_s.tile([128, 2, 512], F32, tag="ps",
                                   name=f"s{sb}_{tb}")
                    for h in (hA, hB):  # disjoint PE row-groups: overlap on HW
                        nc.tensor.matmul(
                            ps[:, h % 2, off:512],
                            lhsT=k_[h][:, s0:s0 + 128],
                            rhs=q_[h][:, t0:tlo + 512],
                            start=True, stop=True)
                    nc.scalar.activation(
                        exp_sb[:, :, sb, slot + off:slot + 512],
                        ps[:, :, off:512],
                        EXP, bias=kb_sb[:, sb:sb + 1], scale=SCALE)
                    if s0 >= tlo:  # diagonal 128-block: causal triangle mask
                        dg = exp_sb[:, :, sb, slot + off:slot + off + 128]
                        nc.vector.tensor_mul(dg, dg, tri_sb)

                def pair(hp, ctx, nxt):
                    # ctx = (wq_jt, wk_jt, qT_t, kT_t); the tb0 q/k chains
                    # were already emitted (previous pair's tb3 prologue or
                    # the caller for pair 0). nxt = next pair's ctx.
                    hA, hB = 2 * hp, 2 * hp + 1
                    wq_jt, wk_jt, qT_t, kT_t = ctx
                    for tb in range(NTB):
                        if tb > 0:
                            qk_chain(wq_jt, qT_t, bq_sb, hp, tb)
                            qk_chain(wk_jt, kT_t, bk_sb, hp, tb)
                        if tb == 3 and nxt is not None:
                            # next pair's first chains fill our tail
                            qk_chain(nxt[0], nxt[2], bq_sb, hp + 1, 0)
                            qk_chain(nxt[1], nxt[3], bk_sb, hp + 1, 0)
                        for sb in range(4 * tb + 4):
                            schunk(hp, qT_t, kT_t, sb, tb)
                        if hp == 0:
                            v_group(4 * tb, 4 * tb + 4)
                        burst(hp, hB, tb)
                        burst(hp, hA, tb)
                        if hp == 3 and tb > 0:
                            # project the PREVIOUS t-block's rows: their yT
                            # finished a whole t-block ago, so no stall on
                            # the recip->broadcast->mul normalize chain
                            out_proj(4 * tb - 4, 4 * tb)

                def out_proj(tt0, tt1):
                    # output projection for t-rows [tt0*128, tt1*128);
                    # emitted right after the pair-3 burst that completes
                    # those yT rows, so it overlaps the rest of pair 3
                    for tt in range(tt0, tt1):
                        for jb in range(2):
                            ps = pp_y.tile([128, 512], F32, tag="py",
                                           name=f"o{tt}{jb}")
                            for cj in range(4):
                                nc.tensor.matmul(
                                    ps,
                                    lhsT=yT_sb[:, cj, tt * 128:(tt + 1) * 128],
                                    rhs=wp_sb[:, cj, jb * 512:(jb + 1) * 512],
                                    start=(cj == 0), stop=(cj == 3))
                            ob = obp.tile([128, 512], F32, tag="ob")
                            nc.any.tensor_copy(out=ob, in_=ps)
                            nc.sync.dma_start(
                                out=part[tt * 128:(tt + 1) * 128,
                                        jb * 512:(jb + 1) * 512],
                                in_=ob)

                def make_ctx(hp):
                    return (load_wjt(wqT, hp, f"wq{hp}"),
                            load_wjt(wkT, hp, f"wk{hp}"),
                            qkp.tile([128, T], BF16, tag="qT", name=f"qT{hp}"),
                            qkp.tile([128, T], BF16, tag="kT", name=f"kT{hp}"))

                ctx = (wq_jt, wk_jt,
                       qkp.tile([128, T], BF16, tag="qT", name="qT0"),
                       qkp.tile([128, T], BF16, tag="kT", name="kT0"))
                qk_chain(ctx[0], ctx[2], bq_sb, 0, 0)
                qk_chain(ctx[1], ctx[3], bk_sb, 0, 0)
                nxt = make_ctx(1)
                pair(0, ctx, nxt)
                for cj in range(4):
                    nc.sync.dma_start(out=wp_sb[:, cj, :],
                                      in_=wpT[cj * 128:(cj + 1) * 128, :])
                for hp in range(1, 4):
                    ctx = nxt
                    nxt = make_ctx(hp + 1) if hp < 3 else None
                    pair(hp, ctx, nxt)
                out_proj(12, 16)

    nc.compile()
    return nc


def _get_nc():
    if "nc" not in _NC_CACHE:
        _NC_CACHE["nc"] = _build()
    return _NC_CACHE["nc"]


def _make_in_maps(x, mask, Wq, bq, Wk, bk, Wv, bv, Wp, bp):
    tri1 = np.triu(np.ones((128, 128), dtype=BF16NP))  # keep s <= t
    tri = np.concatenate([tri1, tri1], axis=1)
    xTs = [np.ascontiguousarray(x[b].T).astype(BF16NP) for b in range(B)]
    kms = [((1.0 - mask[b]) * -10000.0).astype(np.float32) for b in range(B)]
    halves = []
    for half in range(2):
        F = slice(half * NF, half * NF + NF)
        halves.append({
            "wqT": np.ascontiguousarray(Wq[F, :].T).astype(BF16NP),
            "wkT": np.ascontiguousarray(Wk[F, :].T).astype(BF16NP),
            "wvT": np.ascontiguousarray(Wv[F, :].T).astype(BF16NP),
            "wpT": np.ascontiguousarray(Wp[:, F].T).astype(BF16NP),
            "bqv": bq[F].astype(np.float32).copy(),
            "bkv": bk[F].astype(np.float32).copy(),
            "tri": tri,
        })
    return [{"xT": xTs[c // 2], "kmask": kms[c // 2], **halves[c % 2]}
            for c in range(NCORES)]


def kernel(x, mask, Wq, bq, Wk, bk, Wv, bv, Wp, bp):
    x = np.asarray(x, dtype=np.float32)
    mask = np.asarray(mask, dtype=np.float32)
    Wq, bq = np.asarray(Wq, np.float32), np.asarray(bq, np.float32)
    Wk, bk = np.asarray(Wk, np.float32), np.asarray(bk, np.float32)
    Wv, bv = np.asarray(Wv, np.float32), np.asarray(bv, np.float32)
    Wp, bp = np.asarray(Wp, np.float32), np.asarray(bp, np.float32)

    nc = _get_nc()
    in_maps = _make_in_maps(x, mask, Wq, bq, Wk, bk, Wv, bv, Wp, bp)
    res = run_bass_kernel_spmd(nc, in_maps, list(range(NCORES)))
    const_row = (bv @ Wp.T + bp).astype(np.float32)  # [C]
    out = np.empty((B, T, C), np.float32)
    for b in range(B):
        out[b] = res.results[2 * b]["part"] + res.results[2 * b + 1]["part"]
        out[b] += const_row
    return out


# revision 45
# speedup vs baseline: 1.1799x; 1.0924x over previous
"""Causal self-attention kernel for Trainium2, 8 NeuronCores.

Sharding: core c handles batch b = c//2 and head-half c%2 (8 of 16 heads,
512 of 1024 features). Tensor-parallel style: Wq/Wk/Wv split column-wise,
Wp split row-wise; the 2 cores of a batch produce partial outputs that the
host sums (plus the bias const row).

Per-core device program (identical across cores, data differs):
  - projections: qT/kT in [feature, t] layout, v in [s, feature] layout
  - per head-pair (even head on SBUF partitions 0-63, odd head on 64-127 so
    their K=64 score matmuls occupy disjoint PE row-groups and overlap):
    S^T[s, t] = k_s . q_t scores via PE, exp via ACT with fused 1/sqrt(d)
    scale and key-mask bias; causal handled by ragged matmuls plus one
    triangular-mask multiply on the diagonal 128x128 block per chunk.
    Work is tiled as (s-tile 128) x (t-chunk 512); exp rows are stored in
    two 512-wide slots (slot = tb % 2), freed by the att@v burst two
    t-chunks later, so both heads' live scores fit in SBUF.
  - att@v with a ones-column appended to v so the softmax denominator
    accumulates as row 64 of the PSUM tile; normalize by broadcast multiply
    (gpsimd partition_broadcast of the reciprocal row)
  - output projection from the transposed y layout (natural [t, j] output)
"""

import sys

sys.path.insert(0, "/opt/trn_rl_repo")

import numpy as np
import ml_dtypes

import concourse.bass as bass
import concourse.mybir as mybir
import concourse.tile as tile
from concourse import bacc
from concourse.bass_utils import run_bass_kernel_spmd

B, T, C, H = 4, 2048, 1024, 16
D = 64          # head dim
NCORES = 8
NF = 512        # features per core (8 heads)
NH = 8          # heads per core
CT = C // 128   # 8 contraction chunks
NTB = T // 512  # 4 t-blocks
NST = T // 128  # 16 s-tiles
F32 = mybir.dt.float32
BF16 = mybir.dt.bfloat16
BF16NP = ml_dtypes.bfloat16

_NC_CACHE = {}


def _build():
    nc = bacc.Bacc("TRN2", target_bir_lowering=False, debug=False,
                   num_devices=NCORES)
    xT = nc.dram_tensor("xT", [C, T], BF16, kind="ExternalInput")
    wqT = nc.dram_tensor("wqT", [C, NF], BF16, kind="ExternalInput")
    wkT = nc.dram_tensor("wkT", [C, NF], BF16, kind="ExternalInput")
    wvT = nc.dram_tensor("wvT", [C, NF], BF16, kind="ExternalInput")
    wpT = nc.dram_tensor("wpT", [NF, C], BF16, kind="ExternalInput")
    kmask = nc.dram_tensor("kmask", [T], F32, kind="ExternalInput")
    bqv = nc.dram_tensor("bqv", [NF], F32, kind="ExternalInput")
    bkv = nc.dram_tensor("bkv", [NF], F32, kind="ExternalInput")
    tri = nc.dram_tensor("tri", [128, 256], BF16, kind="ExternalInput")
    part = nc.dram_tensor("part", [T, C], F32, kind="ExternalOutput")

    EXP = mybir.ActivationFunctionType.Exp
    SCALE = 1.0 / 8.0  # 1/sqrt(D)

    with tile.TileContext(nc) as tc:
        with (
            tc.tile_pool(name="const", bufs=1) as const,
            tc.tile_pool(name="small", bufs=3) as small, \
            tc.tile_pool(name="obp", bufs=4) as obp,
            tc.tile_pool(name="pp_s", bufs=2, space="PSUM") as pp_s,
            tc.tile_pool(name="pp_y", bufs=2, space="PSUM") as pp_y,
        ):
            # ---- persistent tiles ----
            v_sb = const.tile([128, NST, NH, D + 1], BF16)  # [s_loc, st, h, 65]
            yT_sb = const.tile([128, 4, T], BF16)   # [p, jt, t]
            wp_sb = const.tile([128, 4, C], BF16)   # [p, cj, j]
            tri_sb = const.tile([128, 2, 128], BF16)
            kb_sb = const.tile([128, NST], F32)     # [s_loc, st]
            bq_sb = const.tile([128, 4], F32)
            bk_sb = const.tile([128, 4], F32)

            # ones column for the softmax denominator trick
            nc.vector.memset(v_sb[:, :, :, D:D + 1], 1.0)

            with tc.tile_pool(name="proj", bufs=1) as projp, \
                 tc.tile_pool(name="wjt", bufs=4) as wjtp, \
                 tc.tile_pool(name="qkp", bufs=2) as qkp, \
                 tc.tile_pool(name="wvp", bufs=1) as wvp, \
                 tc.tile_pool(name="expp", bufs=1) as expp:

                def load_wjt(wT, jt, nm):
                    # single 3D-AP DMA: dest [p, ci, q] <- wT[ci*128+p, jt*128+q]
                    w_jt = wjtp.tile([128, CT, 128], BF16, tag="wjt", name=nm)
                    src = bass.AP(tensor=wT, offset=jt * 128,
                                  ap=[[NF, 128], [128 * NF, CT], [1, 128]])
                    nc.sync.dma_start(out=w_jt, in_=src)
                    return w_jt

                x_sb = projp.tile([128, CT, T], BF16)
                for ci in range(2):
                    eng = nc.sync if ci % 2 == 0 else nc.scalar
                    eng.dma_start(out=x_sb[:, ci, 0:1024],
                                  in_=xT[ci * 128:(ci + 1) * 128, 0:1024])
                wq_jt = load_wjt(wqT, 0, "wq0")
                wk_jt = load_wjt(wkT, 0, "wk0")
                nc.sync.dma_start(
                    out=x_sb[:, 2:5, 0:1024],
                    in_=bass.AP(tensor=xT, offset=2 * 128 * T,
                                ap=[[T, 128], [128 * T, 3], [1, 1024]]))
                nc.sync.dma_start(
                    out=x_sb[:, 5:CT, 0:1024],
                    in_=bass.AP(tensor=xT, offset=5 * 128 * T,
                                ap=[[T, 128], [128 * T, CT - 5], [1, 1024]]))
                nc.scalar.dma_start(
                    out=x_sb[:, :, 1024:T],
                    in_=bass.AP(tensor=xT, offset=1024,
                                ap=[[T, 128], [128 * T, CT], [1, 1024]]))
                nc.sync.dma_start(out=tri_sb,
                                  in_=tri.rearrange("p (u q) -> p u q", u=2))
                nc.sync.dma_start(out=kb_sb,
                                  in_=bass.AP(tensor=kmask, offset=0,
                                              ap=[[1, 128], [128, NST]]))
                nc.sync.dma_start(out=bq_sb,
                                  in_=bass.AP(tensor=bqv, offset=0,
                                              ap=[[1, 128], [128, 4]]))
                nc.sync.dma_start(out=bk_sb,
                                  in_=bass.AP(tensor=bkv, offset=0,
                                              ap=[[1, 128], [128, 4]]))
                wv_sb = wvp.tile([128, CT, NF], BF16)
                nc.sync.dma_start(
                    out=wv_sb,
                    in_=bass.AP(tensor=wvT, offset=0,
                                ap=[[NF, 128], [128 * NF, CT], [1, NF]]))

                # [s_loc, head_parity, sb, t_within_phase]
                exp_sb = expp.tile([128, 2, NST, 1024], BF16)

                def qk_chain(w_jt, dst, b_sb, jt, tb):
                    ps = pp_y.tile([128, 512], F32, tag="py", name=f"q{jt}{tb}")
                    for ci in range(CT):
                        nc.tensor.matmul(
                            ps,
                            lhsT=w_jt[:, ci, :],
                            rhs=x_sb[:, ci, tb * 512:(tb + 1) * 512],
                            start=(ci == 0), stop=(ci == CT - 1))
                    nc.vector.tensor_scalar_add(
                        dst[:, tb * 512:(tb + 1) * 512], ps,
                        b_sb[:, jt:jt + 1])

                def v_group(st0, st1):
                    # v: natural [s, feature] layout (bv folded in on host)
                    for st in range(st0, st1):
                        ps = pp_y.tile([128, 512], F32, tag="yb", name=f"v{st}",
                                       bufs=2)
                        for ci in range(CT):
                            nc.tensor.matmul(
                                ps,
                                lhsT=x_sb[:, ci, st * 128:(st + 1) * 128],
                                rhs=wv_sb[:, ci, :],
                                start=(ci == 0), stop=(ci == CT - 1))
                        nc.any.tensor_copy(
                            out=v_sb[:, st, :, 0:D],
                            in_=ps.rearrange("p (h d) -> p h d", h=NH))

                def burst(hp, h, tb):
                    """att @ v for head h, t-block tb (rows 0..4tb+3 ready)."""
                    hh = h % 2
                    tlo = tb * 512
                    slot = (tb % 2) * 512
                    y_ps = pp_y.tile([128, 512], F32, tag="yb", name=f"y{h}{tb}",
                                     bufs=2)
                    smax = 4 * (tb + 1)
                    for i in range(smax):
                        t0 = max(tlo, i * 128)
                        nc.tensor.matmul(
                            y_ps[0:D + 1, t0 - tlo:512],
                            lhsT=v_sb[:, i, h, :],
                            rhs=exp_sb[:, hh, i, slot + t0 - tlo:slot + 512],
                            start=(i == 0), stop=(i == smax - 1))
                    rrow = small.tile([1, 512], F32, tag="rrow")
                    nc.vector.reciprocal(rrow, y_ps[D:D + 1, :])
                    bcast = small.tile([64, 512], F32, tag="bcast")
                    nc.gpsimd.partition_broadcast(bcast, rrow, channels=64)
                    dst_sl = slice(tlo, tlo + 512)
                    if hh == 0:
                        nc.vector.tensor_mul(
                            yT_sb[0:64, hp, dst_sl], y_ps[0:D, :], bcast)
                    else:
                        tmp = small.tile([64, 512], BF16, tag="odd")
                        nc.vector.tensor_mul(tmp, y_ps[0:D, :], bcast)
                        nc.sync.dma_start(
                            out=yT_sb[64:128, hp, dst_sl], in_=tmp)

                def schunk(hp, qT_t, kT_t, sb, tb):
                    """score chunk [s-tile sb] x [t-block tb], both heads in
                    one 2-bank psum tile; one fused exp ACTIVATE."""
                    hA, hB = 2 * hp, 2 * hp + 1
                    q_ = {hA: qT_t[0:64, :], hB: qT_t[64:128, :]}
                    k_ = {hA: kT_t[0:64, :], hB: kT_t[64:128, :]}
                    s0, tlo = sb * 128, tb * 512
                    t0 = max(s0, tlo)
                    off = t0 - tlo
                    slot = (tb % 2) * 512
                    ps = pp_s.tile([128, 2, 512], F32, tag="ps",
                                   name=f"s{sb}_{tb}")
                    for h in (hA, hB):  # disjoint PE row-groups: overlap on HW
                        nc.tensor.matmul(
                            ps[:, h % 2, off:512],
                            lhsT=k_[h][:, s0:s0 + 128],
                            rhs=q_[h][:, t0:tlo + 512],
                            start=True, stop=True)
                    nc.scalar.activation(
                        exp_sb[:, :, sb, slot + off:slot + 512],
                        ps[:, :, off:512],
                        EXP, bias=kb_sb[:, sb:sb + 1], scale=SCALE)
                    if s0 >= tlo:  # diagonal 128-block: causal triangle mask
                        dg = exp_sb[:, :, sb, slot + off:slot + off + 128]
                        nc.vector.tensor_mul(dg, dg, tri_sb)

                def pair(hp, ctx, nxt):
                    # ctx = (wq_jt, wk_jt, qT_t, kT_t); the tb0 q/k chains
                    # were already emitted (previous pair's tb3 prologue or
                    # the caller for pair 0). nxt = next pair's ctx.
                    hA, hB = 2 * hp, 2 * hp + 1
                    wq_jt, wk_jt, qT_t, kT_t = ctx
                    for tb in range(NTB):
                        if tb > 0:
                            qk_chain(wq_jt, qT_t, bq_sb, hp, tb)
                            qk_chain(wk_jt, kT_t, bk_sb, hp, tb)
                        if tb == 3 and nxt is not None:
                            # next pair's first chains fill our tail
                            qk_chain(nxt[0], nxt[2], bq_sb, hp + 1, 0)
                            qk_chain(nxt[1], nxt[3], bk_sb, hp + 1, 0)
                        for sb in range(4 * tb + 4):
                            schunk(hp, qT_t, kT_t, sb, tb)
                        if hp == 0:
                            v_group(4 * tb, 4 * tb + 4)
                        burst(hp, hB, tb)
                        burst(hp, hA, tb)
                        if hp == 3 and tb > 0:
                            # project the PREVIOUS t-block's rows: their yT
                            # finished a whole t-block ago, so no stall on
                            # the recip->broadcast->mul normalize chain
                            out_proj(4 * tb - 4, 4 * tb)

                def out_proj(tt0, tt1):
                    # output projection for t-rows [tt0*128, tt1*128);
                    # emitted right after the pair-3 burst that completes
                    # those yT rows, so it overlaps the rest of pair 3
                    for tt in range(tt0, tt1):
                        for jb in range(2):
                            ps = pp_y.tile([128, 512], F32, tag="py",
                                           name=f"o{tt}{jb}")
                            for cj in range(4):
                                nc.tensor.matmul(
                                    ps,
                                    lhsT=yT_sb[:, cj, tt * 128:(tt + 1) * 128],
                                    rhs=wp_sb[:, cj, jb * 512:(jb + 1) * 512],
                                    start=(cj == 0), stop=(cj == 3))
                            ob = obp.tile([128, 512], F32, tag="ob")
                            nc.any.tensor_copy(out=ob, in_=ps)
                            nc.sync.dma_start(
                                out=part[tt * 128:(tt + 1) * 128,
                                        jb * 512:(jb + 1) * 512],
                                in_=ob)

                def make_ctx(hp):
                    return (load_wjt(wqT, hp, f"wq{hp}"),
                            load_wjt(wkT, hp, f"wk{hp}"),
                            qkp.tile([128, T], BF16, tag="qT", name=f"qT{hp}"),
                            qkp.tile([128, T], BF16, tag="kT", name=f"kT{hp}"))

                ctx = (wq_jt, wk_jt,
                       qkp.tile([128, T], BF16, tag="qT", name="qT0"),
                       qkp.tile([128, T], BF16, tag="kT", name="kT0"))
                qk_chain(ctx[0], ctx[2], bq_sb, 0, 0)
                qk_chain(ctx[1], ctx[3], bk_sb, 0, 0)
                nxt = make_ctx(1)
                pair(0, ctx, nxt)
                nc.sync.dma_start(
                    out=wp_sb,
                    in_=bass.AP(tensor=wpT, offset=0,
                                ap=[[C, 128], [128 * C, 4], [1, C]]))
                for hp in range(1, 4):
                    ctx = nxt
                    nxt = make_ctx(hp + 1) if hp < 3 else None
                    pair(hp, ctx, nxt)
                out_proj(12, 16)

    nc.compile()
    return nc


def _get_nc():
    if "nc" not in _NC_CACHE:
        _NC_CACHE["nc"] = _build()
    return _NC_CACHE["nc"]


def _make_in_maps(x, mask, Wq, bq, Wk, bk, Wv, bv, Wp, bp):
    tri1 = np.triu(np.ones((128, 128), dtype=BF16NP))  # keep s <= t
    tri = np.concatenate([tri1, tri1], axis=1)
    xTs = [np.ascontiguousarray(x[b].T).astype(BF16NP) for b in range(B)]
    kms = [((1.0 - mask[b]) * -10000.0).astype(np.float32) for b in range(B)]
    halves = []
    for half in range(2):
        F = slice(half * NF, half * NF + NF)
        halves.append({
            "wqT": np.ascontiguousarray(Wq[F, :].T).astype(BF16NP),
            "wkT": np.ascontiguousarray(Wk[F, :].T).astype(BF16NP),
            "wvT": np.ascontiguousarray(Wv[F, :].T).astype(BF16NP),
            "wpT": np.ascontiguousarray(Wp[:, F].T).astype(BF16NP),
            "bqv": bq[F].astype(np.float32).copy(),
            "bkv": bk[F].astype(np.float32).copy(),
            "tri": tri,
        })
    return [{"xT": xTs[c // 2], "kmask": kms[c // 2], **halves[c % 2]}
            for c in range(NCORES)]


def kernel(x, mask, Wq, bq, Wk, bk, Wv, bv, Wp, bp):
    x = np.asarray(x, dtype=np.float32)
    mask = np.asarray(mask, dtype=np.float32)
    Wq, bq = np.asarray(Wq, np.float32), np.asarray(bq, np.float32)
    Wk, bk = np.asarray(Wk, np.float32), np.asarray(bk, np.float32)
    Wv, bv = np.asarray(Wv, np.float32), np.asarray(bv, np.float32)
    Wp, bp = np.asarray(Wp, np.float32), np.asarray(bp, np.float32)

    nc = _get_nc()
    in_maps = _make_in_maps(x, mask, Wq, bq, Wk, bk, Wv, bv, Wp, bp)
    res = run_bass_kernel_spmd(nc, in_maps, list(range(NCORES)))
    const_row = (bv @ Wp.T + bp).astype(np.float32)  # [C]
    out = np.empty((B, T, C), np.float32)
    for b in range(B):
        out[b] = res.results[2 * b]["part"] + res.results[2 * b + 1]["part"]
        out[b] += const_row
    return out


# revision 50
# speedup vs baseline: 1.1811x; 1.0010x over previous
"""Causal self-attention kernel for Trainium2, 8 NeuronCores.

Sharding: core c handles batch b = c//2 and head-half c%2 (8 of 16 heads,
512 of 1024 features). Tensor-parallel style: Wq/Wk/Wv split column-wise,
Wp split row-wise; the 2 cores of a batch produce partial outputs that the
host sums (plus the bias const row).

Per-core device program (identical across cores, data differs):
  - projections: qT/kT in [feature, t] layout, v in [s, feature] layout
  - per head-pair (even head on SBUF partitions 0-63, odd head on 64-127 so
    their K=64 score matmuls occupy disjoint PE row-groups and overlap):
    S^T[s, t] = k_s . q_t scores via PE, exp via ACT with fused 1/sqrt(d)
    scale and key-mask bias; causal handled by ragged matmuls plus one
    triangular-mask multiply on the diagonal 128x128 block per chunk.
    Work is tiled as (s-tile 128) x (t-chunk 512); exp rows are stored in
    two 512-wide slots (slot = tb % 2), freed by the att@v burst two
    t-chunks later, so both heads' live scores fit in SBUF.
  - att@v with a ones-column appended to v so the softmax denominator
    accumulates as row 64 of the PSUM tile; normalize by broadcast multiply
    (gpsimd partition_broadcast of the reciprocal row)
  - output projection from the transposed y layout (natural [t, j] output)
"""

import sys

sys.path.insert(0, "/opt/trn_rl_repo")

import numpy as np
import ml_dtypes

import concourse.bass as bass
import concourse.mybir as mybir
import concourse.tile as tile
from concourse import bacc
from concourse.bass_utils import run_bass_kernel_spmd

B, T, C, H = 4, 2048, 1024, 16
D = 64          # head dim
NCORES = 8
NF = 512        # features per core (8 heads)
NH = 8          # heads per core
CT = C // 128   # 8 contraction chunks
NTB = T // 512  # 4 t-blocks
NST = T // 128  # 16 s-tiles
F32 = mybir.dt.float32
BF16 = mybir.dt.bfloat16
BF16NP = ml_dtypes.bfloat16

_NC_CACHE = {}


def _build():
    nc = bacc.Bacc("TRN2", target_bir_lowering=False, debug=False,
                   num_devices=NCORES)
    xT = nc.dram_tensor("xT", [C, T], BF16, kind="ExternalInput")
    wqT = nc.dram_tensor("wqT", [C, NF], BF16, kind="ExternalInput")
    wkT = nc.dram_tensor("wkT", [C, NF], BF16, kind="ExternalInput")
    wvT = nc.dram_tensor("wvT", [C, NF], BF16, kind="ExternalInput")
    wpT = nc.dram_tensor("wpT", [NF, C], BF16, kind="ExternalInput")
    kmask = nc.dram_tensor("kmask", [T], F32, kind="ExternalInput")
    bqv = nc.dram_tensor("bqv", [NF], F32, kind="ExternalInput")
    bkv = nc.dram_tensor("bkv", [NF], F32, kind="ExternalInput")
    tri = nc.dram_tensor("tri", [128, 256], BF16, kind="ExternalInput")
    part = nc.dram_tensor("part", [T, C], F32, kind="ExternalOutput")

    EXP = mybir.ActivationFunctionType.Exp
    SCALE = 1.0 / 8.0  # 1/sqrt(D)

    with tile.TileContext(nc) as tc:
        with (
            tc.tile_pool(name="const", bufs=1) as const,
            tc.tile_pool(name="small", bufs=3) as small, \
            tc.tile_pool(name="obp", bufs=6) as obp,
            tc.tile_pool(name="pp_s", bufs=2, space="PSUM") as pp_s,
            tc.tile_pool(name="pp_y", bufs=2, space="PSUM") as pp_y,
        ):
            # ---- persistent tiles ----
            v_sb = const.tile([128, NST, NH, D + 1], BF16)  # [s_loc, st, h, 65]
            yT_sb = const.tile([128, 4, T], BF16)   # [p, jt, t]
            wp_sb = const.tile([128, 4, C], BF16)   # [p, cj, j]
            tri_sb = const.tile([128, 2, 128], BF16)
            kb_sb = const.tile([128, NST], F32)     # [s_loc, st]
            bq_sb = const.tile([128, 4], F32)
            bk_sb = const.tile([128, 4], F32)

            # ones column for the softmax denominator trick
            nc.vector.memset(v_sb[:, :, :, D:D + 1], 1.0)

            with tc.tile_pool(name="proj", bufs=1) as projp, \
                 tc.tile_pool(name="wjt", bufs=4) as wjtp, \
                 tc.tile_pool(name="qkp", bufs=2) as qkp, \
                 tc.tile_pool(name="wvp", bufs=1) as wvp, \
                 tc.tile_pool(name="expp", bufs=1) as expp:

                def load_wjt(wT, jt, nm):
                    # single 3D-AP DMA: dest [p, ci, q] <- wT[ci*128+p, jt*128+q]
                    w_jt = wjtp.tile([128, CT, 128], BF16, tag="wjt", name=nm)
                    src = bass.AP(tensor=wT, offset=jt * 128,
                                  ap=[[NF, 128], [128 * NF, CT], [1, 128]])
                    nc.sync.dma_start(out=w_jt, in_=src)
                    return w_jt

                x_sb = projp.tile([128, CT, T], BF16)
                for ci in range(2):
                    eng = nc.sync if ci % 2 == 0 else nc.scalar
                    eng.dma_start(out=x_sb[:, ci, 0:1024],
                                  in_=xT[ci * 128:(ci + 1) * 128, 0:1024])
                wq_jt = load_wjt(wqT, 0, "wq0")
                wk_jt = load_wjt(wkT, 0, "wk0")
                nc.sync.dma_start(
                    out=x_sb[:, 2:5, 0:1024],
                    in_=bass.AP(tensor=xT, offset=2 * 128 * T,
                                ap=[[T, 128], [128 * T, 3], [1, 1024]]))
                nc.sync.dma_start(
                    out=x_sb[:, 5:CT, 0:1024],
                    in_=bass.AP(tensor=xT, offset=5 * 128 * T,
                                ap=[[T, 128], [128 * T, CT - 5], [1, 1024]]))
                nc.scalar.dma_start(
                    out=x_sb[:, :, 1024:T],
                    in_=bass.AP(tensor=xT, offset=1024,
                                ap=[[T, 128], [128 * T, CT], [1, 1024]]))
                nc.sync.dma_start(out=tri_sb,
                                  in_=tri.rearrange("p (u q) -> p u q", u=2))
                nc.sync.dma_start(out=kb_sb,
                                  in_=bass.AP(tensor=kmask, offset=0,
                                              ap=[[1, 128], [128, NST]]))
                nc.sync.dma_start(out=bq_sb,
                                  in_=bass.AP(tensor=bqv, offset=0,
                                              ap=[[1, 128], [128, 4]]))
                nc.sync.dma_start(out=bk_sb,
                                  in_=bass.AP(tensor=bkv, offset=0,
                                              ap=[[1, 128], [128, 4]]))
                wv_sb = wvp.tile([128, CT, NF], BF16)
                nc.sync.dma_start(
                    out=wv_sb,
                    in_=bass.AP(tensor=wvT, offset=0,
                                ap=[[NF, 128], [128 * NF, CT], [1, NF]]))

                # [s_loc, head_parity, sb, t_within_phase]
                exp_sb = expp.tile([128, 2, NST, 1024], BF16)

                def qk_chain(w_jt, dst, b_sb, jt, tb):
                    ps = pp_y.tile([128, 512], F32, tag="py", name=f"q{jt}{tb}")
                    for ci in range(CT):
                        nc.tensor.matmul(
                            ps,
                            lhsT=w_jt[:, ci, :],
                            rhs=x_sb[:, ci, tb * 512:(tb + 1) * 512],
                            start=(ci == 0), stop=(ci == CT - 1))
                    nc.vector.tensor_scalar_add(
                        dst[:, tb * 512:(tb + 1) * 512], ps,
                        b_sb[:, jt:jt + 1])

                def v_group(st0, st1):
                    # v: natural [s, feature] layout (bv folded in on host)
                    for st in range(st0, st1):
                        ps = pp_y.tile([128, 512], F32, tag="yb", name=f"v{st}",
                                       bufs=2)
                        for ci in range(CT):
                            nc.tensor.matmul(
                                ps,
                                lhsT=x_sb[:, ci, st * 128:(st + 1) * 128],
                                rhs=wv_sb[:, ci, :],
                                start=(ci == 0), stop=(ci == CT - 1))
                        nc.any.tensor_copy(
                            out=v_sb[:, st, :, 0:D],
                            in_=ps.rearrange("p (h d) -> p h d", h=NH))

                def burst(hp, h, tb):
                    """att @ v for head h, t-block tb (rows 0..4tb+3 ready)."""
                    hh = h % 2
                    tlo = tb * 512
                    slot = (tb % 2) * 512
                    y_ps = pp_y.tile([128, 512], F32, tag="yb", name=f"y{h}{tb}",
                                     bufs=2)
                    smax = 4 * (tb + 1)
                    for i in range(smax):
                        t0 = max(tlo, i * 128)
                        nc.tensor.matmul(
                            y_ps[0:D + 1, t0 - tlo:512],
                            lhsT=v_sb[:, i, h, :],
                            rhs=exp_sb[:, hh, i, slot + t0 - tlo:slot + 512],
                            start=(i == 0), stop=(i == smax - 1))
                    rrow = small.tile([1, 512], F32, tag="rrow")
                    nc.vector.reciprocal(rrow, y_ps[D:D + 1, :])
                    bcast = small.tile([64, 512], F32, tag="bcast")
                    nc.gpsimd.partition_broadcast(bcast, rrow, channels=64)
                    dst_sl = slice(tlo, tlo + 512)
                    if hh == 0:
                        nc.vector.tensor_mul(
                            yT_sb[0:64, hp, dst_sl], y_ps[0:D, :], bcast)
                    else:
                        tmp = small.tile([64, 512], BF16, tag="odd")
                        nc.vector.tensor_mul(tmp, y_ps[0:D, :], bcast)
                        nc.sync.dma_start(
                            out=yT_sb[64:128, hp, dst_sl], in_=tmp)

                def schunk(hp, qT_t, kT_t, sb, tb):
                    """score chunk [s-tile sb] x [t-block tb], both heads in
                    one 2-bank psum tile; one fused exp ACTIVATE."""
                    hA, hB = 2 * hp, 2 * hp + 1
                    q_ = {hA: qT_t[0:64, :], hB: qT_t[64:128, :]}
                    k_ = {hA: kT_t[0:64, :], hB: kT_t[64:128, :]}
                    s0, tlo = sb * 128, tb * 512
                    t0 = max(s0, tlo)
                    off = t0 - tlo
                    slot = (tb % 2) * 512
                    ps = pp_s.tile([128, 2, 512], F32, tag="ps",
                                   name=f"s{sb}_{tb}")
                    for h in (hA, hB):  # disjoint PE row-groups: overlap on HW
                        nc.tensor.matmul(
                            ps[:, h % 2, off:512],
                            lhsT=k_[h][:, s0:s0 + 128],
                            rhs=q_[h][:, t0:tlo + 512],
                            start=True, stop=True)
                    nc.scalar.activation(
                        exp_sb[:, :, sb, slot + off:slot + 512],
                        ps[:, :, off:512],
                        EXP, bias=kb_sb[:, sb:sb + 1], scale=SCALE)
                    if s0 >= tlo:  # diagonal 128-block: causal triangle mask
                        dg = exp_sb[:, :, sb, slot + off:slot + off + 128]
                        nc.vector.tensor_mul(dg, dg, tri_sb)

                def pair(hp, ctx, nxt):
                    # ctx = (wq_jt, wk_jt, qT_t, kT_t); the tb0 q/k chains
                    # were already emitted (previous pair's tb3 prologue or
                    # the caller for pair 0). nxt = next pair's ctx.
                    hA, hB = 2 * hp, 2 * hp + 1
                    wq_jt, wk_jt, qT_t, kT_t = ctx
                    for tb in range(NTB):
                        if tb > 0:
                            qk_chain(wq_jt, qT_t, bq_sb, hp, tb)
                            qk_chain(wk_jt, kT_t, bk_sb, hp, tb)
                        if tb == 3 and nxt is not None:
                            # next pair's first chains fill our tail
                            qk_chain(nxt[0], nxt[2], bq_sb, hp + 1, 0)
                            qk_chain(nxt[1], nxt[3], bk_sb, hp + 1, 0)
                        for sb in range(4 * tb + 4):
                            schunk(hp, qT_t, kT_t, sb, tb)
                        if hp == 0:
                            v_group(4 * tb, 4 * tb + 4)
                        burst(hp, hB, tb)
                        burst(hp, hA, tb)
                        if hp == 3 and tb > 0:
                            # project the PREVIOUS t-block's rows: their yT
                            # finished a whole t-block ago, so no stall on
                            # the recip->broadcast->mul normalize chain
                            out_proj(4 * tb - 4, 4 * tb)

                def out_proj(tt0, tt1):
                    # output projection for t-rows [tt0*128, tt1*128);
                    # emitted right after the pair-3 burst that completes
                    # those yT rows, so it overlaps the rest of pair 3
                    for tt in range(tt0, tt1):
                        for jb in range(2):
                            ps = pp_y.tile([128, 512], F32, tag="py",
                                           name=f"o{tt}{jb}")
                            for cj in range(4):
                                nc.tensor.matmul(
                                    ps,
                                    lhsT=yT_sb[:, cj, tt * 128:(tt + 1) * 128],
                                    rhs=wp_sb[:, cj, jb * 512:(jb + 1) * 512],
                                    start=(cj == 0), stop=(cj == 3))
                            ob = obp.tile([128, 512], F32, tag="ob")
                            nc.any.tensor_copy(out=ob, in_=ps)
                            nc.sync.dma_start(
                                out=part[tt * 128:(tt + 1) * 128,
                                        jb * 512:(jb + 1) * 512],
                                in_=ob)

                def make_ctx(hp):
                    return (load_wjt(wqT, hp, f"wq{hp}"),
                            load_wjt(wkT, hp, f"wk{hp}"),
                            qkp.tile([128, T], BF16, tag="qT", name=f"qT{hp}"),
                            qkp.tile([128, T], BF16, tag="kT", name=f"kT{hp}"))

                ctx = (wq_jt, wk_jt,
                       qkp.tile([128, T], BF16, tag="qT", name="qT0"),
                       qkp.tile([128, T], BF16, tag="kT", name="kT0"))
                qk_chain(ctx[0], ctx[2], bq_sb, 0, 0)
                qk_chain(ctx[1], ctx[3], bk_sb, 0, 0)
                nxt = make_ctx(1)
                pair(0, ctx, nxt)
                nc.sync.dma_start(
                    out=wp_sb,
                    in_=bass.AP(tensor=wpT, offset=0,
                                ap=[[C, 128], [128 * C, 4], [1, C]]))
                for hp in range(1, 4):
                    ctx = nxt
                    nxt = make_ctx(hp + 1) if hp < 3 else None
                    pair(hp, ctx, nxt)
                out_proj(12, 16)

    nc.compile()
    return nc


def _get_nc():
    if "nc" not in _NC_CACHE:
        _NC_CACHE["nc"] = _build()
    return _NC_CACHE["nc"]


def _make_in_maps(x, mask, Wq, bq, Wk, bk, Wv, bv, Wp, bp):
    tri1 = np.triu(np.ones((128, 128), dtype=BF16NP))  # keep s <= t
    tri = np.concatenate([tri1, tri1], axis=1)
    xTs = [np.ascontiguousarray(x[b].T).astype(BF16NP) for b in range(B)]
    kms = [((1.0 - mask[b]) * -10000.0).astype(np.float32) for b in range(B)]
    halves = []
    for half in range(2):
        F = slice(half * NF, half * NF + NF)
        halves.append({
            "wqT": np.ascontiguousarray(Wq[F, :].T).astype(BF16NP),
            "wkT": np.ascontiguousarray(Wk[F, :].T).astype(BF16NP),
            "wvT": np.ascontiguousarray(Wv[F, :].T).astype(BF16NP),
            "wpT": np.ascontiguousarray(Wp[:, F].T).astype(BF16NP),
            "bqv": bq[F].astype(np.float32).copy(),
            "bkv": bk[F].astype(np.float32).copy(),
            "tri": tri,
        })
    return [{"xT": xTs[c // 2], "kmask": kms[c // 2], **halves[c % 2]}
            for c in range(NCORES)]


def kernel(x, mask, Wq, bq, Wk, bk, Wv, bv, Wp, bp):
    x = np.asarray(x, dtype=np.float32)
    mask = np.asarray(mask, dtype=np.float32)
    Wq, bq = np.asarray(Wq, np.float32), np.asarray(bq, np.float32)
    Wk, bk = np.asarray(Wk, np.float32), np.asarray(bk, np.float32)
    Wv, bv = np.asarray(Wv, np.float32), np.asarray(bv, np.float32)
    Wp, bp = np.asarray(Wp, np.float32), np.asarray(bp, np.float32)

    nc = _get_nc()
    in_maps = _make_in_maps(x, mask, Wq, bq, Wk, bk, Wv, bv, Wp, bp)
    res = run_bass_kernel_spmd(nc, in_maps, list(range(NCORES)))
    const_row = (bv @ Wp.T + bp).astype(np.float32)  # [C]
    out = np.empty((B, T, C), np.float32)
    for b in range(B):
        out[b] = res.results[2 * b]["part"] + res.results[2 * b + 1]["part"]
        out[b] += const_row
    return out


# revision 53
# speedup vs baseline: 1.1878x; 1.0057x over previous
"""Causal self-attention kernel for Trainium2, 8 NeuronCores.

Sharding: core c handles batch b = c//2 and head-half c%2 (8 of 16 heads,
512 of 1024 features). Tensor-parallel style: Wq/Wk/Wv split column-wise,
Wp split row-wise; the 2 cores of a batch produce partial outputs that the
host sums (plus the bias const row).

Per-core device program (identical across cores, data differs):
  - projections: qT/kT in [feature, t] layout, v in [s, feature] layout
  - per head-pair (even head on SBUF partitions 0-63, odd head on 64-127 so
    their K=64 score matmuls occupy disjoint PE row-groups and overlap):
    S^T[s, t] = k_s . q_t scores via PE, exp via ACT with fused 1/sqrt(d)
    scale and key-mask bias; causal handled by ragged matmuls plus one
    triangular-mask multiply on the diagonal 128x128 block per chunk.
    Work is tiled as (s-tile 128) x (t-chunk 512); exp rows are stored in
    two 512-wide slots (slot = tb % 2), freed by the att@v burst two
    t-chunks later, so both heads' live scores fit in SBUF.
  - att@v with a ones-column appended to v so the softmax denominator
    accumulates as row 64 of the PSUM tile; normalize by broadcast multiply
    (gpsimd partition_broadcast of the reciprocal row)
  - output projection from the transposed y layout (natural [t, j] output)
"""

import sys

sys.path.insert(0, "/opt/trn_rl_repo")

import numpy as np
import ml_dtypes

import concourse.bass as bass
import concourse.mybir as mybir
import concourse.tile as tile
from concourse import bacc
from concourse.bass_utils import run_bass_kernel_spmd

B, T, C, H = 4, 2048, 1024, 16
D = 64          # head dim
NCORES = 8
NF = 512        # features per core (8 heads)
NH = 8          # heads per core
CT = C // 128   # 8 contraction chunks
NTB = T // 512  # 4 t-blocks
NST = T // 128  # 16 s-tiles
F32 = mybir.dt.float32
BF16 = mybir.dt.bfloat16
BF16NP = ml_dtypes.bfloat16

_NC_CACHE = {}


def _build():
    nc = bacc.Bacc("TRN2", target_bir_lowering=False, debug=False,
                   num_devices=NCORES)
    xT = nc.dram_tensor("xT", [C, T], BF16, kind="ExternalInput")
    # weights arrive pre-packed in SBUF-tile layout (contiguous DMAs):
    # wqP/wkP[jt*128+p, ci*128+q], wvP[p, ci*512+f], wpP[p, cj*1024+j]
    wqP = nc.dram_tensor("wqP", [NF, C], BF16, kind="ExternalInput")
    wkP = nc.dram_tensor("wkP", [NF, C], BF16, kind="ExternalInput")
    wvP = nc.dram_tensor("wvP", [128, CT * NF], BF16, kind="ExternalInput")
    wpP = nc.dram_tensor("wpP", [128, 4 * C], BF16, kind="ExternalInput")
    kmask = nc.dram_tensor("kmask", [128, NST], F32, kind="ExternalInput")
    bqv = nc.dram_tensor("bqv", [128, 4], F32, kind="ExternalInput")
    bkv = nc.dram_tensor("bkv", [128, 4], F32, kind="ExternalInput")
    tri = nc.dram_tensor("tri", [128, 256], BF16, kind="ExternalInput")
    part = nc.dram_tensor("part", [T, C], F32, kind="ExternalOutput")

    EXP = mybir.ActivationFunctionType.Exp
    SCALE = 1.0 / 8.0  # 1/sqrt(D)

    with tile.TileContext(nc) as tc:
        with (
            tc.tile_pool(name="const", bufs=1) as const,
            tc.tile_pool(name="small", bufs=3) as small, \
            tc.tile_pool(name="obp", bufs=6) as obp,
            tc.tile_pool(name="pp_s", bufs=2, space="PSUM") as pp_s,
            tc.tile_pool(name="pp_y", bufs=2, space="PSUM") as pp_y,
        ):
            # ---- persistent tiles ----
            v_sb = const.tile([128, NST, NH, D + 1], BF16)  # [s_loc, st, h, 65]
            yT_sb = const.tile([128, 4, T], BF16)   # [p, jt, t]
            wp_sb = const.tile([128, 4, C], BF16)   # [p, cj, j]
            tri_sb = const.tile([128, 2, 128], BF16)
            kb_sb = const.tile([128, NST], F32)     # [s_loc, st]
            bq_sb = const.tile([128, 4], F32)
            bk_sb = const.tile([128, 4], F32)

            # ones column for the softmax denominator trick
            nc.vector.memset(v_sb[:, :, :, D:D + 1], 1.0)

            with tc.tile_pool(name="proj", bufs=1) as projp, \
                 tc.tile_pool(name="wjt", bufs=4) as wjtp, \
                 tc.tile_pool(name="qkp", bufs=2) as qkp, \
                 tc.tile_pool(name="wvp", bufs=1) as wvp, \
                 tc.tile_pool(name="expp", bufs=1) as expp:

                def load_wjt(wP, jt, nm):
                    # contiguous 2D DMA from the host-packed layout
                    w_jt = wjtp.tile([128, CT, 128], BF16, tag="wjt", name=nm)
                    nc.sync.dma_start(
                        out=w_jt,
                        in_=wP[jt * 128:(jt + 1) * 128, :].rearrange(
                            "p (ci q) -> p ci q", ci=CT))
                    return w_jt

                x_sb = projp.tile([128, CT, T], BF16)
                for ci in range(2):
                    eng = nc.sync if ci % 2 == 0 else nc.scalar
                    eng.dma_start(out=x_sb[:, ci, 0:1024],
                                  in_=xT[ci * 128:(ci + 1) * 128, 0:1024])
                wq_jt = load_wjt(wqP, 0, "wq0")
                wk_jt = load_wjt(wkP, 0, "wk0")
                nc.sync.dma_start(
                    out=x_sb[:, 2:5, 0:1024],
                    in_=bass.AP(tensor=xT, offset=2 * 128 * T,
                                ap=[[T, 128], [128 * T, 3], [1, 1024]]))
                nc.sync.dma_start(
                    out=x_sb[:, 5:CT, 0:1024],
                    in_=bass.AP(tensor=xT, offset=5 * 128 * T,
                                ap=[[T, 128], [128 * T, CT - 5], [1, 1024]]))
                nc.scalar.dma_start(
                    out=x_sb[:, :, 1024:T],
                    in_=bass.AP(tensor=xT, offset=1024,
                                ap=[[T, 128], [128 * T, CT], [1, 1024]]))
                nc.sync.dma_start(out=tri_sb,
                                  in_=tri.rearrange("p (u q) -> p u q", u=2))
                nc.sync.dma_start(out=kb_sb, in_=kmask.ap())
                nc.sync.dma_start(out=bq_sb, in_=bqv.ap())
                nc.sync.dma_start(out=bk_sb, in_=bkv.ap())
                wv_sb = wvp.tile([128, CT, NF], BF16)
                nc.sync.dma_start(
                    out=wv_sb,
                    in_=wvP.rearrange("p (ci f) -> p ci f", ci=CT))

                # [s_loc, head_parity, sb, t_within_phase]
                exp_sb = expp.tile([128, 2, NST, 1024], BF16)

                def qk_chain(w_jt, dst, b_sb, jt, tb):
                    ps = pp_y.tile([128, 512], F32, tag="py", name=f"q{jt}{tb}")
                    for ci in range(CT):
                        nc.tensor.matmul(
                            ps,
                            lhsT=w_jt[:, ci, :],
                            rhs=x_sb[:, ci, tb * 512:(tb + 1) * 512],
                            start=(ci == 0), stop=(ci == CT - 1))
                    nc.vector.tensor_scalar_add(
                        dst[:, tb * 512:(tb + 1) * 512], ps,
                        b_sb[:, jt:jt + 1])

                def v_group(st0, st1):
                    # v: natural [s, feature] layout (bv folded in on host)
                    for st in range(st0, st1):
                        ps = pp_y.tile([128, 512], F32, tag="yb", name=f"v{st}",
                                       bufs=2)
                        for ci in range(CT):
                            nc.tensor.matmul(
                                ps,
                                lhsT=x_sb[:, ci, st * 128:(st + 1) * 128],
                                rhs=wv_sb[:, ci, :],
                                start=(ci == 0), stop=(ci == CT - 1))
                        nc.any.tensor_copy(
                            out=v_sb[:, st, :, 0:D],
                            in_=ps.rearrange("p (h d) -> p h d", h=NH))

                def burst(hp, h, tb):
                    """att @ v for head h, t-block tb (rows 0..4tb+3 ready)."""
                    hh = h % 2
                    tlo = tb * 512
                    slot = (tb % 2) * 512
                    y_ps = pp_y.tile([128, 512], F32, tag="yb", name=f"y{h}{tb}",
                                     bufs=2)
                    smax = 4 * (tb + 1)
                    for i in range(smax):
                        t0 = max(tlo, i * 128)
                        nc.tensor.matmul(
                            y_ps[0:D + 1, t0 - tlo:512],
                            lhsT=v_sb[:, i, h, :],
                            rhs=exp_sb[:, hh, i, slot + t0 - tlo:slot + 512],
                            start=(i == 0), stop=(i == smax - 1))
                    rrow = small.tile([1, 512], F32, tag="rrow")
                    nc.vector.reciprocal(rrow, y_ps[D:D + 1, :])
                    bcast = small.tile([64, 512], F32, tag="bcast")
                    nc.gpsimd.partition_broadcast(bcast, rrow, channels=64)
                    dst_sl = slice(tlo, tlo + 512)
                    if hh == 0:
                        nc.vector.tensor_mul(
                            yT_sb[0:64, hp, dst_sl], y_ps[0:D, :], bcast)
                    else:
                        tmp = small.tile([64, 512], BF16, tag="odd")
                        nc.vector.tensor_mul(tmp, y_ps[0:D, :], bcast)
                        nc.sync.dma_start(
                            out=yT_sb[64:128, hp, dst_sl], in_=tmp)

                def schunk(hp, qT_t, kT_t, sb, tb):
                    """score chunk [s-tile sb] x [t-block tb], both heads in
                    one 2-bank psum tile; one fused exp ACTIVATE."""
                    hA, hB = 2 * hp, 2 * hp + 1
                    q_ = {hA: qT_t[0:64, :], hB: qT_t[64:128, :]}
                    k_ = {hA: kT_t[0:64, :], hB: kT_t[64:128, :]}
                    s0, tlo = sb * 128, tb * 512
                    t0 = max(s0, tlo)
                    off = t0 - tlo
                    slot = (tb % 2) * 512
                    ps = pp_s.tile([128, 2, 512], F32, tag="ps",
                                   name=f"s{sb}_{tb}")
                    for h in (hA, hB):  # disjoint PE row-groups: overlap on HW
                        nc.tensor.matmul(
                            ps[:, h % 2, off:512],
                            lhsT=k_[h][:, s0:s0 + 128],
                            rhs=q_[h][:, t0:tlo + 512],
                            start=True, stop=True)
                    nc.scalar.activation(
                        exp_sb[:, :, sb, slot + off:slot + 512],
                        ps[:, :, off:512],
                        EXP, bias=kb_sb[:, sb:sb + 1], scale=SCALE)
                    if s0 >= tlo:  # diagonal 128-block: causal triangle mask
                        dg = exp_sb[:, :, sb, slot + off:slot + off + 128]
                        nc.vector.tensor_mul(dg, dg, tri_sb)

                def pair(hp, ctx, nxt):
                    # ctx = (wq_jt, wk_jt, qT_t, kT_t); the tb0 q/k chains
                    # were already emitted (previous pair's tb3 prologue or
                    # the caller for pair 0). nxt = next pair's ctx.
                    hA, hB = 2 * hp, 2 * hp + 1
                    wq_jt, wk_jt, qT_t, kT_t = ctx
                    for tb in range(NTB):
                        if tb > 0:
                            qk_chain(wq_jt, qT_t, bq_sb, hp, tb)
                            qk_chain(wk_jt, kT_t, bk_sb, hp, tb)
                        if tb == 3 and nxt is not None:
                            # next pair's first chains fill our tail
                            qk_chain(nxt[0], nxt[2], bq_sb, hp + 1, 0)
                            qk_chain(nxt[1], nxt[3], bk_sb, hp + 1, 0)
                        for sb in range(4 * tb + 4):
                            schunk(hp, qT_t, kT_t, sb, tb)
                        if hp == 0:
                            v_group(4 * tb, 4 * tb + 4)
                        burst(hp, hB, tb)
                        burst(hp, hA, tb)
                        if hp == 3 and tb > 0:
                            # project the PREVIOUS t-block's rows: their yT
                            # finished a whole t-block ago, so no stall on
                            # the recip->broadcast->mul normalize chain
                            out_proj(4 * tb - 4, 4 * tb)

                def out_proj(tt0, tt1):
                    # output projection for t-rows [tt0*128, tt1*128);
                    # emitted right after the pair-3 burst that completes
                    # those yT rows, so it overlaps the rest of pair 3
                    for tt in range(tt0, tt1):
                        for jb in range(2):
                            ps = pp_y.tile([128, 512], F32, tag="py",
                                           name=f"o{tt}{jb}")
                            for cj in range(4):
                                nc.tensor.matmul(
                                    ps,
                                    lhsT=yT_sb[:, cj, tt * 128:(tt + 1) * 128],
                                    rhs=wp_sb[:, cj, jb * 512:(jb + 1) * 512],
                                    start=(cj == 0), stop=(cj == 3))
                            ob = obp.tile([128, 512], F32, tag="ob")
                            nc.any.tensor_copy(out=ob, in_=ps)
                            nc.sync.dma_start(
                                out=part[tt * 128:(tt + 1) * 128,
                                        jb * 512:(jb + 1) * 512],
                                in_=ob)

                def make_ctx(hp):
                    return (load_wjt(wqP, hp, f"wq{hp}"),
                            load_wjt(wkP, hp, f"wk{hp}"),
                            qkp.tile([128, T], BF16, tag="qT", name=f"qT{hp}"),
                            qkp.tile([128, T], BF16, tag="kT", name=f"kT{hp}"))

                ctx = (wq_jt, wk_jt,
                       qkp.tile([128, T], BF16, tag="qT", name="qT0"),
                       qkp.tile([128, T], BF16, tag="kT", name="kT0"))
                qk_chain(ctx[0], ctx[2], bq_sb, 0, 0)
                qk_chain(ctx[1], ctx[3], bk_sb, 0, 0)
                nxt = make_ctx(1)
                pair(0, ctx, nxt)
                nc.sync.dma_start(
                    out=wp_sb,
                    in_=wpP.rearrange("p (cj j) -> p cj j", cj=4))
                for hp in range(1, 4):
                    ctx = nxt
                    nxt = make_ctx(hp + 1) if hp < 3 else None
                    pair(hp, ctx, nxt)
                out_proj(12, 16)

    nc.compile()
    return nc


def _get_nc():
    if "nc" not in _NC_CACHE:
        _NC_CACHE["nc"] = _build()
    return _NC_CACHE["nc"]


def _make_in_maps(x, mask, Wq, bq, Wk, bk, Wv, bv, Wp, bp):
    tri1 = np.triu(np.ones((128, 128), dtype=BF16NP))  # keep s <= t
    tri = np.concatenate([tri1, tri1], axis=1)
    xTs = [np.ascontiguousarray(x[b].T).astype(BF16NP) for b in range(B)]
    kms = [np.ascontiguousarray(((1.0 - mask[b]) * -10000.0)
                                .astype(np.float32).reshape(NST, 128).T)
           for b in range(B)]
    halves = []

    def pack_qk(W, F):
        # [jt*128+p, ci*128+q] <- W[F][jt*128+q, ci*128+p]
        wT = W[F, :].T.astype(BF16NP)            # [C(ci p), NF(jt q)]
        a = wT.reshape(CT, 128, 4, 128)          # [ci, p, jt, q]
        return np.ascontiguousarray(
            a.transpose(2, 1, 0, 3).reshape(NF, C))

    for half in range(2):
        F = slice(half * NF, half * NF + NF)
        wvT = Wv[F, :].T.astype(BF16NP)          # [C, NF]
        wvPk = np.ascontiguousarray(
            wvT.reshape(CT, 128, NF).transpose(1, 0, 2).reshape(128, CT * NF))
        wpT = Wp[:, F].T.astype(BF16NP)          # [NF, C]
        wpPk = np.ascontiguousarray(
            wpT.reshape(4, 128, C).transpose(1, 0, 2).reshape(128, 4 * C))
        halves.append({
            "wqP": pack_qk(Wq, F),
            "wkP": pack_qk(Wk, F),
            "wvP": wvPk,
            "wpP": wpPk,
            "bqv": np.ascontiguousarray(
                bq[F].astype(np.float32).reshape(4, 128).T),
            "bkv": np.ascontiguousarray(
                bk[F].astype(np.float32).reshape(4, 128).T),
            "tri": tri,
        })
    return [{"xT": xTs[c // 2], "kmask": kms[c // 2], **halves[c % 2]}
            for c in range(NCORES)]


def kernel(x, mask, Wq, bq, Wk, bk, Wv, bv, Wp, bp):
    x = np.asarray(x, dtype=np.float32)
    mask = np.asarray(mask, dtype=np.float32)
    Wq, bq = np.asarray(Wq, np.float32), np.asarray(bq, np.float32)
    Wk, bk = np.asarray(Wk, np.float32), np.asarray(bk, np.float32)
    Wv, bv = np.asarray(Wv, np.float32), np.asarray(bv, np.float32)
    Wp, bp = np.asarray(Wp, np.float32), np.asarray(bp, np.float32)

    nc = _get_nc()
    in_maps = _make_in_maps(x, mask, Wq, bq, Wk, bk, Wv, bv, Wp, bp)
    res = run_bass_kernel_spmd(nc, in_maps, list(range(NCORES)))
    const_row = (bv @ Wp.T + bp).astype(np.float32)  # [C]
    out = np.empty((B, T, C), np.float32)
    for b in range(B):
        out[b] = res.results[2 * b]["part"] + res.results[2 * b + 1]["part"]
        out[b] += const_row
    return out


# revision 59
# speedup vs baseline: 1.1901x; 1.0020x over previous
"""Causal self-attention kernel for Trainium2, 8 NeuronCores.

Sharding: core c handles batch b = c//2 and head-half c%2 (8 of 16 heads,
512 of 1024 features). Tensor-parallel style: Wq/Wk/Wv split column-wise,
Wp split row-wise; the 2 cores of a batch produce partial outputs that the
host sums (plus the bias const row).

Per-core device program (identical across cores, data differs):
  - projections: qT/kT in [feature, t] layout, v in [s, feature] layout
  - per head-pair (even head on SBUF partitions 0-63, odd head on 64-127 so
    their K=64 score matmuls occupy disjoint PE row-groups and overlap):
    S^T[s, t] = k_s . q_t scores via PE, exp via ACT with fused 1/sqrt(d)
    scale and key-mask bias; causal handled by ragged matmuls plus one
    triangular-mask multiply on the diagonal 128x128 block per chunk.
    Work is tiled as (s-tile 128) x (t-chunk 512); exp rows are stored in
    two 512-wide slots (slot = tb % 2), freed by the att@v burst two
    t-chunks later, so both heads' live scores fit in SBUF.
  - att@v with a ones-column appended to v so the softmax denominator
    accumulates as row 64 of the PSUM tile; normalize by broadcast multiply
    (gpsimd partition_broadcast of the reciprocal row)
  - output projection from the transposed y layout (natural [t, j] output)
"""

import sys

sys.path.insert(0, "/opt/trn_rl_repo")

import numpy as np
import ml_dtypes

import concourse.bass as bass
import concourse.mybir as mybir
import concourse.tile as tile
from concourse import bacc
from concourse.bass_utils import run_bass_kernel_spmd

B, T, C, H = 4, 2048, 1024, 16
D = 64          # head dim
NCORES = 8
NF = 512        # features per core (8 heads)
NH = 8          # heads per core
CT = C // 128   # 8 contraction chunks
NTB = T // 512  # 4 t-blocks
NST = T // 128  # 16 s-tiles
F32 = mybir.dt.float32
BF16 = mybir.dt.bfloat16
BF16NP = ml_dtypes.bfloat16

_NC_CACHE = {}


def _build():
    nc = bacc.Bacc("TRN2", target_bir_lowering=False, debug=False,
                   num_devices=NCORES)
    xT = nc.dram_tensor("xT", [C, T], BF16, kind="ExternalInput")
    # weights arrive pre-packed in SBUF-tile layout (contiguous DMAs):
    # wqP/wkP[jt*128+p, ci*128+q], wvP[p, ci*512+f], wpP[p, cj*1024+j]
    wqP = nc.dram_tensor("wqP", [NF, C], BF16, kind="ExternalInput")
    wkP = nc.dram_tensor("wkP", [NF, C], BF16, kind="ExternalInput")
    wvP = nc.dram_tensor("wvP", [128, CT * NF], BF16, kind="ExternalInput")
    wpP = nc.dram_tensor("wpP", [128, 4 * C], BF16, kind="ExternalInput")
    cst = nc.dram_tensor("cst", [128, NST + 8], F32, kind="ExternalInput")
    tri = nc.dram_tensor("tri", [128, 256], BF16, kind="ExternalInput")
    part = nc.dram_tensor("part", [T, C], F32, kind="ExternalOutput")

    EXP = mybir.ActivationFunctionType.Exp
    SCALE = 1.0 / 8.0  # 1/sqrt(D)

    with tile.TileContext(nc) as tc:
        with (
            tc.tile_pool(name="const", bufs=1) as const,
            tc.tile_pool(name="small", bufs=3) as small, \
            tc.tile_pool(name="obp", bufs=6) as obp,
            tc.tile_pool(name="pp_s", bufs=2, space="PSUM") as pp_s,
            tc.tile_pool(name="pp_y", bufs=2, space="PSUM") as pp_y,
        ):
            # ---- persistent tiles ----
            v_sb = const.tile([128, NST, NH, D + 1], BF16)  # [s_loc, st, h, 65]
            yT_sb = const.tile([128, 4, T], BF16)   # [p, jt, t]
            wp_sb = const.tile([128, 4, C], BF16)   # [p, cj, j]
            tri_sb = const.tile([128, 2, 128], BF16)
            cst_sb = const.tile([128, NST + 8], F32)  # kb[16] | bq[4] | bk[4]

            # ones column for the softmax denominator trick
            nc.vector.memset(v_sb[:, :, :, D:D + 1], 1.0)

            with tc.tile_pool(name="proj", bufs=1) as projp, \
                 tc.tile_pool(name="wjt", bufs=4) as wjtp, \
                 tc.tile_pool(name="qkp", bufs=2) as qkp, \
                 tc.tile_pool(name="wvp", bufs=1) as wvp, \
                 tc.tile_pool(name="expp", bufs=1) as expp:

                def load_wjt(wP, jt, nm):
                    # contiguous 2D DMA from the host-packed layout
                    w_jt = wjtp.tile([128, CT, 128], BF16, tag="wjt", name=nm)
                    nc.sync.dma_start(
                        out=w_jt,
                        in_=wP[jt * 128:(jt + 1) * 128, :].rearrange(
                            "p (ci q) -> p ci q", ci=CT))
                    return w_jt

                x_sb = projp.tile([128, CT, T], BF16)
                for ci in range(2):
                    eng = nc.sync if ci % 2 == 0 else nc.scalar
                    eng.dma_start(out=x_sb[:, ci, 0:1024],
                                  in_=xT[ci * 128:(ci + 1) * 128, 0:1024])
                wq_jt = load_wjt(wqP, 0, "wq0")
                wk_jt = load_wjt(wkP, 0, "wk0")
                nc.sync.dma_start(
                    out=x_sb[:, 2:5, 0:1024],
                    in_=bass.AP(tensor=xT, offset=2 * 128 * T,
                                ap=[[T, 128], [128 * T, 3], [1, 1024]]))
                nc.sync.dma_start(
                    out=x_sb[:, 5:CT, 0:1024],
                    in_=bass.AP(tensor=xT, offset=5 * 128 * T,
                                ap=[[T, 128], [128 * T, CT - 5], [1, 1024]]))
                nc.scalar.dma_start(
                    out=x_sb[:, :, 1024:T],
                    in_=bass.AP(tensor=xT, offset=1024,
                                ap=[[T, 128], [128 * T, CT], [1, 1024]]))
                nc.sync.dma_start(out=tri_sb,
                                  in_=tri.rearrange("p (u q) -> p u q", u=2))
                nc.sync.dma_start(out=cst_sb, in_=cst.ap())
                wv_sb = wvp.tile([128, CT, NF], BF16)
                nc.sync.dma_start(
                    out=wv_sb,
                    in_=wvP.rearrange("p (ci f) -> p ci f", ci=CT))

                # [s_loc, head_parity, sb, t_within_phase]
                exp_sb = expp.tile([128, 2, NST, 1024], BF16)

                def qk_chain(w_jt, dst, bcol, jt, tb):
                    ps = pp_y.tile([128, 512], F32, tag="py", name=f"q{jt}{tb}")
                    for ci in range(CT):
                        nc.tensor.matmul(
                            ps,
                            lhsT=w_jt[:, ci, :],
                            rhs=x_sb[:, ci, tb * 512:(tb + 1) * 512],
                            start=(ci == 0), stop=(ci == CT - 1))
                    nc.vector.tensor_scalar_add(
                        dst[:, tb * 512:(tb + 1) * 512], ps,
                        cst_sb[:, bcol + jt:bcol + jt + 1])

                def v_group(st0, st1):
                    # v: natural [s, feature] layout (bv folded in on host)
                    for st in range(st0, st1):
                        ps = pp_y.tile([128, 512], F32, tag="yb", name=f"v{st}",
                                       bufs=2)
                        for ci in range(CT):
                            nc.tensor.matmul(
                                ps,
                                lhsT=x_sb[:, ci, st * 128:(st + 1) * 128],
                                rhs=wv_sb[:, ci, :],
                                start=(ci == 0), stop=(ci == CT - 1))
                        nc.any.tensor_copy(
                            out=v_sb[:, st, :, 0:D],
                            in_=ps.rearrange("p (h d) -> p h d", h=NH))

                def burst(hp, h, tb):
                    """att @ v for head h, t-block tb (rows 0..4tb+3 ready)."""
                    hh = h % 2
                    tlo = tb * 512
                    slot = (tb % 2) * 512
                    y_ps = pp_y.tile([128, 512], F32, tag="yb", name=f"y{h}{tb}",
                                     bufs=2)
                    smax = 4 * (tb + 1)
                    for i in range(smax):
                        t0 = max(tlo, i * 128)
                        nc.tensor.matmul(
                            y_ps[0:D + 1, t0 - tlo:512],
                            lhsT=v_sb[:, i, h, :],
                            rhs=exp_sb[:, hh, i, slot + t0 - tlo:slot + 512],
                            start=(i == 0), stop=(i == smax - 1))
                    rrow = small.tile([1, 512], F32, tag="rrow")
                    nc.vector.reciprocal(rrow, y_ps[D:D + 1, :])
                    bcast = small.tile([64, 512], F32, tag="bcast")
                    nc.gpsimd.partition_broadcast(bcast, rrow, channels=64)
                    dst_sl = slice(tlo, tlo + 512)
                    if hh == 0:
                        nc.vector.tensor_mul(
                            yT_sb[0:64, hp, dst_sl], y_ps[0:D, :], bcast)
                    else:
                        tmp = small.tile([64, 512], BF16, tag="odd")
                        nc.vector.tensor_mul(tmp, y_ps[0:D, :], bcast)
                        nc.sync.dma_start(
                            out=yT_sb[64:128, hp, dst_sl], in_=tmp)

                def schunk(hp, qT_t, kT_t, sb, tb):
                    """score chunk [s-tile sb] x [t-block tb], both heads in
                    one 2-bank psum tile; one fused exp ACTIVATE."""
                    hA, hB = 2 * hp, 2 * hp + 1
                    q_ = {hA: qT_t[0:64, :], hB: qT_t[64:128, :]}
                    k_ = {hA: kT_t[0:64, :], hB: kT_t[64:128, :]}
                    s0, tlo = sb * 128, tb * 512
                    t0 = max(s0, tlo)
                    off = t0 - tlo
                    slot = (tb % 2) * 512
                    ps = pp_s.tile([128, 2, 512], F32, tag="ps",
                                   name=f"s{sb}_{tb}")
                    for h in (hA, hB):  # disjoint PE row-groups: overlap on HW
                        nc.tensor.matmul(
                            ps[:, h % 2, off:512],
                            lhsT=k_[h][:, s0:s0 + 128],
                            rhs=q_[h][:, t0:tlo + 512],
                            start=True, stop=True)
                    nc.scalar.activation(
                        exp_sb[:, :, sb, slot + off:slot + 512],
                        ps[:, :, off:512],
                        EXP, bias=cst_sb[:, sb:sb + 1], scale=SCALE)
                    if s0 >= tlo:  # diagonal 128-block: causal triangle mask
                        dg = exp_sb[:, :, sb, slot + off:slot + off + 128]
                        nc.vector.tensor_mul(dg, dg, tri_sb)

                def pair(hp, ctx, nxt):
                    # ctx = (wq_jt, wk_jt, qT_t, kT_t); the tb0 q/k chains
                    # were already emitted (previous pair's tb3 prologue or
                    # the caller for pair 0). nxt = next pair's ctx.
                    hA, hB = 2 * hp, 2 * hp + 1
                    wq_jt, wk_jt, qT_t, kT_t = ctx
                    for tb in range(NTB):
                        if tb > 0:
                            qk_chain(wq_jt, qT_t, 16, hp, tb)
                            qk_chain(wk_jt, kT_t, 20, hp, tb)
                        if tb == 3 and nxt is not None:
                            # next pair's first chains fill our tail
                            qk_chain(nxt[0], nxt[2], 16, hp + 1, 0)
                            qk_chain(nxt[1], nxt[3], 20, hp + 1, 0)
                        for sb in range(4 * tb + 4):
                            schunk(hp, qT_t, kT_t, sb, tb)
                        if hp == 0:
                            v_group(4 * tb, 4 * tb + 4)
                        burst(hp, hB, tb)
                        burst(hp, hA, tb)
                        if hp == 3 and tb > 0:
                            # project the PREVIOUS t-block's rows: their yT
                            # finished a whole t-block ago, so no stall on
                            # the recip->broadcast->mul normalize chain
                            out_proj(4 * tb - 4, 4 * tb)

                def out_proj(tt0, tt1):
                    # output projection for t-rows [tt0*128, tt1*128);
                    # emitted right after the pair-3 burst that completes
                    # those yT rows, so it overlaps the rest of pair 3
                    for tt in range(tt0, tt1):
                        for jb in range(2):
                            ps = pp_y.tile([128, 512], F32, tag="py",
                                           name=f"o{tt}{jb}")
                            for cj in range(4):
                                nc.tensor.matmul(
                                    ps,
                                    lhsT=yT_sb[:, cj, tt * 128:(tt + 1) * 128],
                                    rhs=wp_sb[:, cj, jb * 512:(jb + 1) * 512],
                                    start=(cj == 0), stop=(cj == 3))
                            ob = obp.tile([128, 512], F32, tag="ob")
                            nc.any.tensor_copy(out=ob, in_=ps)
                            nc.sync.dma_start(
                                out=part[tt * 128:(tt + 1) * 128,
                                        jb * 512:(jb + 1) * 512],
                                in_=ob)

                def make_ctx(hp):
                    return (load_wjt(wqP, hp, f"wq{hp}"),
                            load_wjt(wkP, hp, f"wk{hp}"),
                            qkp.tile([128, T], BF16, tag="qT", name=f"qT{hp}"),
                            qkp.tile([128, T], BF16, tag="kT", name=f"kT{hp}"))

                ctx = (wq_jt, wk_jt,
                       qkp.tile([128, T], BF16, tag="qT", name="qT0"),
                       qkp.tile([128, T], BF16, tag="kT", name="kT0"))
                qk_chain(ctx[0], ctx[2], 16, 0, 0)
                qk_chain(ctx[1], ctx[3], 20, 0, 0)
                nxt = make_ctx(1)
                pair(0, ctx, nxt)
                nc.sync.dma_start(
                    out=wp_sb,
                    in_=wpP.rearrange("p (cj j) -> p cj j", cj=4))
                for hp in range(1, 4):
                    ctx = nxt
                    nxt = make_ctx(hp + 1) if hp < 3 else None
                    pair(hp, ctx, nxt)
                out_proj(12, 16)

    nc.compile()
    return nc


def _get_nc():
    if "nc" not in _NC_CACHE:
        _NC_CACHE["nc"] = _build()
    return _NC_CACHE["nc"]


def _make_in_maps(x, mask, Wq, bq, Wk, bk, Wv, bv, Wp, bp):
    tri1 = np.triu(np.ones((128, 128), dtype=BF16NP))  # keep s <= t
    tri = np.concatenate([tri1, tri1], axis=1)
    xTs = [np.ascontiguousarray(x[b].T).astype(BF16NP) for b in range(B)]
    def pack_cst(b, F0, F1):
        kb = ((1.0 - mask[b]) * -10000.0).astype(np.float32).reshape(NST, 128).T
        return np.ascontiguousarray(np.concatenate(
            [kb, bq[F0:F1].astype(np.float32).reshape(4, 128).T,
             bk[F0:F1].astype(np.float32).reshape(4, 128).T], axis=1))
    halves = []

    def pack_qk(W, F):
        # [jt*128+p, ci*128+q] <- W[F][jt*128+q, ci*128+p]
        wT = W[F, :].T.astype(BF16NP)            # [C(ci p), NF(jt q)]
        a = wT.reshape(CT, 128, 4, 128)          # [ci, p, jt, q]
        return np.ascontiguousarray(
            a.transpose(2, 1, 0, 3).reshape(NF, C))

    for half in range(2):
        F = slice(half * NF, half * NF + NF)
        wvT = Wv[F, :].T.astype(BF16NP)          # [C, NF]
        wvPk = np.ascontiguousarray(
            wvT.reshape(CT, 128, NF).transpose(1, 0, 2).reshape(128, CT * NF))
        wpT = Wp[:, F].T.astype(BF16NP)          # [NF, C]
        wpPk = np.ascontiguousarray(
            wpT.reshape(4, 128, C).transpose(1, 0, 2).reshape(128, 4 * C))
        halves.append({
            "wqP": pack_qk(Wq, F),
            "wkP": pack_qk(Wk, F),
            "wvP": wvPk,
            "wpP": wpPk,
            "tri": tri,
        })
    return [{"xT": xTs[c // 2],
             "cst": pack_cst(c // 2, (c % 2) * NF, (c % 2) * NF + NF),
             **halves[c % 2]}
            for c in range(NCORES)]


def kernel(x, mask, Wq, bq, Wk, bk, Wv, bv, Wp, bp):
    x = np.asarray(x, dtype=np.float32)
    mask = np.asarray(mask, dtype=np.float32)
    Wq, bq = np.asarray(Wq, np.float32), np.asarray(bq, np.float32)
    Wk, bk = np.asarray(Wk, np.float32), np.asarray(bk, np.float32)
    Wv, bv = np.asarray(Wv, np.float32), np.asarray(bv, np.float32)
    Wp, bp = np.asarray(Wp, np.float32), np.asarray(bp, np.float32)

    nc = _get_nc()
    in_maps = _make_in_maps(x, mask, Wq, bq, Wk, bk, Wv, bv, Wp, bp)
    res = run_bass_kernel_spmd(nc, in_maps, list(range(NCORES)))
    const_row = (bv @ Wp.T + bp).astype(np.float32)  # [C]
    out = np.empty((B, T, C), np.float32)
    for b in range(B):
        out[b] = res.results[2 * b]["part"] + res.results[2 * b + 1]["part"]
        out[b] += const_row
    return out
